# revision 37
# baseline (speedup 1.0000x reference)
"""Cross-attention Trainium2 kernel (8 NeuronCores, SPMD).

Sharding: core c handles batch c//2 and head-group c%2 (8 of 16 heads).
Each core computes its head-group's partial output projection; the host
sums the two partials per batch (bias is folded into head-group 0).

Design (cost-model driven; see transcript):
- All inputs arrive HOST-TRANSPOSED in the exact SBUF layout (k-major),
  with fp8e4m3 main+residual pairs packed in one tensor per operand
  ([P, 2, KO, C]; dim1 = {e4m3(s*a), e4m3(s*a - rounded)}), so the
  critical first-exp DMA path is 4 transfers.
- Q/K/V/O projections run as fp8 DoubleRow with 3-chain residual
  compensation (a8@b8 + da8@b8 + a8@db8) accumulated in one PSUM tile:
  4x faster per chain than fp16, 3 chains -> 1.33x net, ~fp16 accuracy.
- QK^T runs as fp8 DoubleRow on 32-partition row tiles (contraction
  64 = 32 partitions x 2 half-head k-tiles interleaved in the free
  dim): 2x over fp16.  Q/K are requantized to e4m3 (x16) from the
  projection PSUM; the 1/256 descale folds into the exp scale.
- exp on ACT is the critical engine (~133us busy).  Emission is
  unit-granular: after every S PSUM tile (2 chunks + exp) the PE pops
  ~0.9us of queued work (PV chains, projection sub-chains, out-proj)
  so ACT never starves and the PE p-state stays warm (the cost model
  halves PE speed after ~3.4us of idle; dummy warm-up matmuls cover
  the DMA-bound head).
- PV stays fp16 (plain fp8 fails the 2e-2 gate).  Softmax row sums
  ride as a ones column in V; DVE normalizes with reciprocals.
- O round-trip: O_sb -> DRAM scratch -> XBAR transpose per 128-query
  block -> compensated-fp8 out-proj, interleaved into later slots.
  The last query chunk skips the DRAM round-trip entirely: each
  finished head pair is PE-transposed (identity matmul) straight into
  the fp8 out-proj operands, removing two serial DMA hops per block
  from the kernel tail.
"""
import sys

if "/opt/trn_rl_repo" not in sys.path:
    sys.path.insert(0, "/opt/trn_rl_repo")

from collections import deque

import numpy as np
import ml_dtypes

import concourse.bass as bass  # noqa: F401
import concourse.tile as tile
from concourse import bacc, mybir
from concourse.bass_utils import run_bass_kernel_spmd

P = 128
N = 2048          # queries per batch
M = 1024          # context rows
K = 1024          # query_dim == context_dim
DHG = 512         # d_attn per head group (8 heads x 64)
DH = 64           # dim per head
HL = 8            # heads per core
E = 1024          # output dim
SCALE = DH ** -0.5
QS = 16.0         # fp8 scale for q8/k8 and oT8
XS = 4.0          # host fp8 scale for x/ctx
WS = 64.0         # host fp8 scale for weights
F32 = mybir.dt.float32
F16 = mybir.dt.float16
E4 = mybir.dt.float8e4
E4NP = ml_dtypes.float8_e4m3

KO = K // P       # 8 contraction chunks
KP = KO // 2      # 4 DoubleRow contraction pairs
MT = M // P       # 8 context tiles
DO = DHG // P     # 4 head-dim chunks
QC = N // 512     # 4 query chunks of 512
NC = 512 // P     # 4 query sub-tiles per chunk
EC = E // 512     # 2 output chunks of 512

DR = mybir.MatmulPerfMode.DoubleRow
MUL = mybir.AluOpType.mult
ADD = mybir.AluOpType.add
SUB = mybir.AluOpType.subtract

_CACHE = {}


def _build():
    nc = bacc.Bacc("TRN2", target_bir_lowering=False, debug=False, num_devices=8)
    xp_d = nc.dram_tensor("xp", [P, 2, KO, N], E4, kind="ExternalInput")
    cp_d = nc.dram_tensor("cp", [P, 2, KO, M], E4, kind="ExternalInput")
    wqp_d = nc.dram_tensor("wqp", [2, P, 2, 2, KO, P], E4, kind="ExternalInput")
    wkp_d = nc.dram_tensor("wkp", [2, P, 2, 2, KO, P], E4, kind="ExternalInput")
    wvp_d = nc.dram_tensor("wvp", [P, 2, KO, DHG], E4, kind="ExternalInput")
    wop_d = nc.dram_tensor("wop", [P, 2, DO, E], E4, kind="ExternalInput")
    ident_d = nc.dram_tensor("ident", [P, P], F16, kind="ExternalInput")
    bo_d = nc.dram_tensor("bo", [1, E], F32, kind="ExternalInput")
    out_d = nc.dram_tensor("out", [N, E], F16, kind="ExternalOutput")
    oscr_d = nc.dram_tensor("oscr", [N, DHG], F16, kind="Internal")

    with tile.TileContext(nc) as tc:
        with tc.tile_pool(name="persist", bufs=1) as pp, \
             tc.tile_pool(name="ptp", bufs=8) as ptp, \
             tc.tile_pool(name="osb", bufs=2) as osb, \
             tc.tile_pool(name="otp", bufs=8) as otp, \
             tc.tile_pool(name="odp", bufs=4) as odp, \
             tc.tile_pool(name="psS", bufs=2, space="PSUM") as psS, \
             tc.tile_pool(name="psV", bufs=2, space="PSUM") as psV, \
             tc.tile_pool(name="psF", bufs=2, space="PSUM") as psF:
            xp = pp.tile([P, 2, KO, N], E4)
            cp = pp.tile([P, 2, KO, M], E4)
            wqp = [pp.tile([P, 2, 2, KO, P], E4, name=f"wq_{c}") for c in range(2)]
            wkp = [pp.tile([P, 2, 2, KO, P], E4, name=f"wk_{c}") for c in range(2)]
            wvp = pp.tile([P, 2, KO, DHG], E4)
            wop = pp.tile([P, 2, DO, E], E4)
            ident = pp.tile([P, P], F16)
            scratch = pp.tile([P, 512], F16)
            bo_sb = pp.tile([1, E], F32)
            bias_sb = pp.tile([P, E], F32)
            # fp8 Q/K, half-head interleaved: [p=(h4,dh), hg, half, n]
            q8 = pp.tile([P, 2, 2, N], E4)
            k8 = pp.tile([P, 2, 2, M], E4)
            v_sb = pp.tile([P, MT, HL, DH + 1], F16)
            rec_sb = pp.tile([P, QC, HL, NC], F32)

            # ---------------- DMA loads -------------------------------
            # Critical path to the first exp: K/Q mains first, residuals
            # right behind, everything else after.
            nc.sync.dma_start(wkp[0][:], wkp_d[0])
            nc.sync.dma_start(cp[:, 0, :, 0:512], cp_d[:, 0, :, 0:512])
            nc.sync.dma_start(wqp[0][:], wqp_d[0])
            nc.sync.dma_start(xp[:, 0, :, 0:512], xp_d[:, 0, :, 0:512])
            nc.sync.dma_start(cp[:, 1, :, 0:512], cp_d[:, 1, :, 0:512])
            nc.sync.dma_start(xp[:, 1, :, 0:512], xp_d[:, 1, :, 0:512])
            nc.sync.dma_start(cp[:, 0, :, 512:M], cp_d[:, 0, :, 512:M])
            nc.sync.dma_start(cp[:, 1, :, 512:M], cp_d[:, 1, :, 512:M])
            nc.sync.dma_start(wkp[1][:], wkp_d[1])
            nc.sync.dma_start(wqp[1][:], wqp_d[1])
            nc.sync.dma_start(wvp[:], wvp_d[:])
            for qc in range(1, QC):
                nc.sync.dma_start(xp[:, :, :, qc * 512:(qc + 1) * 512],
                                  xp_d[:, :, :, qc * 512:(qc + 1) * 512])
            nc.sync.dma_start(wop[:], wop_d[:])
            nc.sync.dma_start(bo_sb[:], bo_d[:])
            nc.sync.dma_start(ident[:], ident_d[:])
            nc.gpsimd.partition_broadcast(bias_sb[:], bo_sb[:])
            nc.vector.memset(v_sb[:, :, :, DH], 1.0)
            nc.vector.memset(scratch[:], 0.0)

            # PE p-state warm-up / keep-warm dummies (the cost model halves
            # matmul speed unless the PE has been continuously busy ~3us).
            wid = [0]

            def warm(n):
                for _ in range(n):
                    s = psS.tile([P, 2, 512], F32, tag="s",
                                 name=f"warm_{wid[0]}")
                    wid[0] += 1
                    nc.tensor.matmul(
                        s[:, 0], scratch[:, 0:P], scratch[:],
                        start=True, stop=True, skip_group_check=True)

            # ------------- compensated fp8 projection chains ----------
            def sub_chain(ps, lhs, rhs, first, last):
                for kp in range(KP):
                    nc.tensor.matmul(
                        ps, lhs(kp), rhs(kp),
                        start=(first and kp == 0), stop=(last and kp == KP - 1),
                        perf_mode=DR,
                    )

            def qk_chain_units(name, wt, xt, cc, x0, finish, pool=None):
                cell = {}
                wc = wt[cc // 256]
                c2 = (cc // P) % 2
                wl = lambda r: (lambda kp: wc[:, c2, r, 2 * kp:2 * kp + 2, :])
                xr = lambda r: (lambda kp: xt[:, r, 2 * kp:2 * kp + 2, x0:x0 + 512])

                def u1():
                    if pool is None:
                        cell["ps"] = psF.tile([P, 512], F32, tag="f", name=name)
                    else:
                        # phase-A only: borrow an S-pool tile so the four
                        # head-of-kernel chains don't serialize on psF bufs
                        cell["ps"] = psS.tile([P, 2, 512], F32, tag="s",
                                              name=name)[:, 0, :]
                    sub_chain(cell["ps"][:], wl(0), xr(0), True, False)
                u2 = lambda: sub_chain(cell["ps"][:], wl(1), xr(0), False, False)

                def u3():
                    sub_chain(cell["ps"][:], wl(0), xr(1), False, True)
                    finish(cell["ps"])
                return [(430, u1), (430, u2), (470, u3)]

            def q_units(qc, hg, half, pool=None):
                cc = (hg * 2 + half) * P

                def fin(ps):
                    nc.vector.tensor_scalar(
                        q8[:, hg, half, qc * 512:(qc + 1) * 512], ps[:],
                        QS / (XS * WS), None, MUL)
                return qk_chain_units(f"qc_{qc}{hg}{half}", wqp, xp,
                                      cc, qc * 512, fin, pool=pool)

            def k_units(hg, half, ms):
                cc = (hg * 2 + half) * P

                def fin(ps):
                    nc.vector.tensor_scalar(
                        k8[:, hg, half, ms * 512:(ms + 1) * 512], ps[:],
                        QS / (XS * WS), None, MUL)
                return qk_chain_units(f"kc_{hg}{half}{ms}", wkp, cp,
                                      cc, ms * 512, fin)

            def v_units(mo):
                cell = {}
                cl = lambda r: (lambda kp: cp[:, r, 2 * kp:2 * kp + 2,
                                              mo * P:(mo + 1) * P])
                wr = lambda r: (lambda kp: wvp[:, r, 2 * kp:2 * kp + 2, :])

                def u1():
                    cell["ps"] = psF.tile([P, 512], F32, tag="f", name=f"vc_{mo}")
                    sub_chain(cell["ps"][:], cl(0), wr(0), True, False)
                u2 = lambda: sub_chain(cell["ps"][:], cl(1), wr(0), False, False)

                def u3():
                    ps = cell["ps"]
                    sub_chain(ps[:], cl(0), wr(1), False, True)
                    nc.vector.tensor_scalar(
                        v_sb[:, mo, :, 0:DH],
                        ps[:].rearrange("p (h d) -> p h d", h=HL),
                        1.0 / (XS * WS), None, MUL)
                return [(430, u1), (430, u2), (470, u3)]

            # ---------------- attention -------------------------------
            def s_tile(qc, h, mp, ptile):
                hg, pb = h // 4, (h % 4) * 32
                q0 = qc * 512
                s = psS.tile([P, 2, 512], F32, tag="s", name=f"s_{qc}_{h}_{mp}")
                for k2 in range(2):
                    mo = 2 * mp + k2
                    nc.tensor.matmul(
                        s[:, k2],
                        k8[pb:pb + 32, hg, :, mo * P:(mo + 1) * P],
                        q8[pb:pb + 32, hg, :, q0:q0 + 512],
                        start=True, stop=True,
                        perf_mode=DR,
                        tile_position=(pb, 0),
                        skip_group_check=True,
                    )
                nc.scalar.activation(
                    ptile[:, 2 * mp:2 * mp + 2, :], s[:],
                    mybir.ActivationFunctionType.Exp,
                    scale=SCALE / (QS * QS),
                )

            O_tiles = {}

            def pv_units(qc, h, ptile):
                cell = {}

                def mk(nci):
                    def u():
                        if nci == 0:
                            cell["pv"] = psV.tile([P, NC, DH + 1], F32,
                                                  tag="pv", name=f"pv_{qc}_{h}")
                        pv = cell["pv"]
                        for mo in range(MT):
                            nc.tensor.matmul(
                                pv[:, nci, :],
                                ptile[:, mo, nci * P:(nci + 1) * P],
                                v_sb[:, mo, h, :],
                                start=(mo == 0), stop=(mo == MT - 1),
                                skip_group_check=True,
                            )
                        if nci == NC - 1:
                            rec = rec_sb[:, qc, h, :]
                            nc.vector.reciprocal(rec, pv[:, :, DH])
                            O_sb = O_tiles[qc]
                            for i in range(NC):
                                nc.vector.tensor_scalar(
                                    O_sb[:, h, i, :], pv[:, i, 0:DH],
                                    rec[:, i:i + 1], None, MUL)
                            if qc != QC - 1:
                                q0 = qc * 512
                                nc.sync.dma_start(
                                    oscr_d[q0:q0 + 512, h * DH:(h + 1) * DH]
                                    .rearrange("(a pn) c -> pn a c", pn=P),
                                    O_sb[:, h])
                    return u
                return [(220, mk(0)), (220, mk(1)), (220, mk(2)), (500, mk(3))]

            def oproj_tiles(qc, nci):
                return (
                    otp.tile([P, DO, P], E4, tag="ot8", name=f"oT8_{qc}_{nci}"),
                    otp.tile([P, DO, P], E4, tag="dot8", name=f"doT8_{qc}_{nci}"),
                    odp.tile([P, E], F16, tag="od", name=f"od_{qc}_{nci}"),
                )

            def oproj_chain_units(qc, nci, oT8, doT8, od):
                """Compensated fp8 out-proj: oT8@wo8 + doT8@wo8 + oT8@dwo8."""
                last = qc == QC - 1
                q0 = qc * 512

                def mk(ec):
                    def u():
                        ps = psF.tile([P, 512], F32, tag="f",
                                      name=f"f_{qc}_{nci}_{ec}")
                        wr = lambda r: (lambda dp: wop[:, r, 2 * dp:2 * dp + 2,
                                                       ec * 512:(ec + 1) * 512])
                        ol = lambda t: (lambda dp: t[:, 2 * dp:2 * dp + 2, :])
                        for ci, (lt, rr) in enumerate(
                                [(oT8, 0), (doT8, 0), (oT8, 1)]):
                            for dp in range(DO // 2):
                                nc.tensor.matmul(
                                    ps[:], ol(lt)(dp), wr(rr)(dp),
                                    start=(ci == 0 and dp == 0),
                                    stop=(ci == 2 and dp == DO // 2 - 1),
                                    perf_mode=DR)
                        nc.vector.scalar_tensor_tensor(
                            od[:, ec * 512:(ec + 1) * 512], ps[:],
                            1.0 / (QS * WS),
                            bias_sb[:, ec * 512:(ec + 1) * 512], MUL, ADD)
                        if last:
                            nc.sync.dma_start(
                                out_d[q0 + nci * P:q0 + (nci + 1) * P,
                                      ec * 512:(ec + 1) * 512],
                                od[:, ec * 512:(ec + 1) * 512])
                        elif ec == EC - 1:
                            nc.gpsimd.dma_start(
                                out_d[q0 + nci * P:q0 + (nci + 1) * P, :], od[:])
                    return u
                return [(680, mk(0)), (700, mk(1))]

            # ---------------- schedule --------------------------------
            urgent = deque()
            background = deque()
            state = {"v_left": MT}

            def emit_budget(ns):
                spent = 0
                while spent < ns and (urgent or background):
                    est, u = urgent.popleft() if urgent else background.popleft()
                    u()
                    spent += est
                if spent == 0:
                    # queues dry: keep the PE p-state warm
                    warm(1)

            def push_pv(qc, h, pt):
                urgent.extend(pv_units(qc, h, pt))
                last = qc == QC - 1
                if h == HL - 1 and not last:
                    def tail():
                        q0 = qc * 512
                        for nci in range(NC):
                            oT = otp.tile([P, DO, P], F16, tag="ot",
                                          name=f"oT_{qc}_{nci}")
                            nc.sync.dma_start_transpose(
                                oT[:], oscr_d[q0 + nci * P:q0 + (nci + 1) * P, :])
                            oT8, doT8, od = oproj_tiles(qc, nci)

                            def conv(oT=oT, oT8=oT8, doT8=doT8):
                                nc.vector.tensor_scalar(
                                    oT8[:], oT[:], QS, None, MUL)
                                nc.vector.scalar_tensor_tensor(
                                    doT8[:], oT[:], QS, oT8[:], MUL, SUB)
                            background.append((100, conv))
                            background.extend(
                                oproj_chain_units(qc, nci, oT8, doT8, od))
                    urgent.append((0, tail))
                if last and h in (1, 3, 5):
                    # qc3 skips the DRAM round-trip entirely: PE-transpose
                    # each finished head pair straight into oT8/doT8.
                    d = h // 2

                    def tp_nci(nci, d=d):
                        def u():
                            if d == 0:
                                state[f"o3_{nci}"] = oproj_tiles(qc, nci)
                            oT8, doT8, _ = state[f"o3_{nci}"]
                            ps = psF.tile([P, 512], F32, tag="f",
                                          name=f"tp{d}_{nci}")
                            pv16 = ps[:].bitcast(F16)
                            nc.tensor.transpose(
                                pv16[0:64, 0:P],
                                O_tiles[qc][:, 2 * d, nci, :], ident[:],
                                tile_position=(0, 0))
                            nc.tensor.transpose(
                                pv16[64:128, 0:P],
                                O_tiles[qc][:, 2 * d + 1, nci, :], ident[:],
                                tile_position=(0, 64))
                            nc.vector.tensor_scalar(
                                oT8[:, d, :], pv16[:, 0:P], QS, None, MUL)
                            nc.vector.scalar_tensor_tensor(
                                doT8[:, d, :], pv16[:, 0:P], QS,
                                oT8[:, d, :], MUL, SUB)
                        return u
                    for nci in range(NC):
                        urgent.append((250, tp_nci(nci)))
                if last and h == HL - 2:
                    def tail6():
                        for nci in range(NC):
                            oT8, doT8, _ = state[f"o3_{nci}"]
                            ps = psF.tile([P, 512], F32, tag="f",
                                          name=f"t6_{nci}")
                            pv16 = ps[:].bitcast(F16)
                            nc.tensor.transpose(
                                pv16[0:64, 0:P],
                                O_tiles[qc][:, 6, nci, :], ident[:],
                                tile_position=(0, 0))
                            nc.vector.tensor_scalar(
                                oT8[0:64, 3, :], pv16[0:64, 0:P],
                                QS, None, MUL)
                            nc.vector.scalar_tensor_tensor(
                                doT8[0:64, 3, :], pv16[0:64, 0:P], QS,
                                oT8[0:64, 3, :], MUL, SUB)
                    urgent.append((300, tail6))
                if last and h == HL - 1:
                    def tail7():
                        for nci in range(NC):
                            oT8, doT8, _ = state[f"o3_{nci}"]
                            ps = psF.tile([P, 512], F32, tag="f",
                                          name=f"t7_{nci}")
                            pv16 = ps[:].bitcast(F16)
                            nc.tensor.transpose(
                                pv16[64:128, 0:P],
                                O_tiles[qc][:, 7, nci, :], ident[:],
                                tile_position=(0, 64))
                            nc.vector.tensor_scalar(
                                oT8[64:128, 3, :], pv16[64:128, 0:P],
                                QS, None, MUL)
                            nc.vector.scalar_tensor_tensor(
                                doT8[64:128, 3, :], pv16[64:128, 0:P], QS,
                                oT8[64:128, 3, :], MUL, SUB)
                        for nci in range(NC):
                            oT8, doT8, od = state[f"o3_{nci}"]
                            urgent.extend(
                                oproj_chain_units(qc, nci, oT8, doT8, od))
                    urgent.append((0, tail7))

            def mk_v(mo):
                def f():
                    for est, u in v_units(mo):
                        u()
                    state["v_left"] -= 1
                return (1330, f)

            # phase A: minimum work before the first exp.  Sub-chain order
            # tracks DMA arrival: mains (u1, u2 use the weight pair + the
            # x/ctx main half), then the x/ctx-residual chains (u3).
            warm(8)
            ka, kb = k_units(0, 0, 0), k_units(0, 1, 0)
            qa, qb = q_units(0, 0, 0, pool="s"), q_units(0, 0, 1, pool="s")
            for est, u in [ka[0], ka[1], kb[0], kb[1]]:
                u()
                warm(1)
            for est, u in [qa[0], qa[1], qb[0], qb[1], ka[2], kb[2],
                           qa[2], qb[2]]:
                u()
            O_tiles[0] = osb.tile([P, HL, NC, DH], F16, tag="o", name="O_0")
            pt00 = ptp.tile([P, MT, 512], F16, tag="pt", name="pt_0_0")
            s_tile(0, 0, 0, pt00)
            for est, u in k_units(0, 0, 1) + k_units(0, 1, 1):
                u()
            s_tile(0, 0, 1, pt00)

            def marker(key):
                return (0, lambda: state.__setitem__(key, True))

            background.extend(k_units(1, 0, 0) + k_units(1, 1, 0))
            background.extend(q_units(0, 1, 0) + q_units(0, 1, 1))
            background.extend(k_units(1, 0, 1) + k_units(1, 1, 1))
            background.append(marker("hg1"))
            background.extend([mk_v(mo) for mo in range(MT)])

            def drain_until(key):
                while not state.get(key) and (urgent or background):
                    emit_budget(1)

            pv_pending = deque([(0, 0, pt00)])
            s_tile(0, 0, 2, pt00)
            emit_budget(900)
            s_tile(0, 0, 3, pt00)
            emit_budget(900)

            for s in range(1, QC * HL):
                qc, h = divmod(s, HL)
                if h == 0:
                    O_tiles[qc] = osb.tile([P, HL, NC, DH], F16, tag="o",
                                           name=f"O_{qc}")
                if h == 1 and qc + 1 < QC:
                    for hg in range(2):
                        for hf in range(2):
                            background.extend(q_units(qc + 1, hg, hf))
                    background.append(marker(f"q{qc + 1}"))
                # S(0, h>=4) needs the hg1 K/Q chains; S(qc, 0) needs the
                # q8 chains of qc -- force-drain them if the budget lagged.
                if qc == 0 and h == 4:
                    drain_until("hg1")
                if h == 0 and qc >= 1:
                    drain_until(f"q{qc}")
                pt = ptp.tile([P, MT, 512], F16, tag="pt", name=f"pt_{qc}_{h}")
                pv_pending.append((qc, h, pt))
                lag = 2 if s < 24 else 1
                while len(pv_pending) > lag and state["v_left"] == 0:
                    push_pv(*pv_pending.popleft())
                for mp in range(4):
                    s_tile(qc, h, mp, pt)
                    emit_budget(980)

            while pv_pending:
                push_pv(*pv_pending.popleft())
            while urgent or background:
                emit_budget(10000)
    nc.finalize()
    return nc


def _get_nc():
    if "nc" not in _CACHE:
        _CACHE["nc"] = _build()
    return _CACHE["nc"]


# column permutation for Wq/Wk: chain-major [hg, half, h4, dh] ordering
def _qk_perm():
    j = np.arange(DHG)
    hg, r = j // 256, j % 256
    half, r2 = r // 128, r % 128
    h4, dh = r2 // 32, r2 % 32
    return hg * 256 + h4 * 64 + half * 32 + dh


_PERM = _qk_perm()


def _pair(a, scale, ko, p):
    """[K, C] -> [P, 2, KO, C]: {e4m3(s*a), residual} in SBUF layout."""
    s = (np.asarray(a, dtype=np.float32) * scale)
    hi = s.astype(E4NP)
    lo = (s - hi.astype(np.float32)).astype(E4NP)
    both = np.stack([hi, lo], axis=0)           # [2, K, C]
    both = both.reshape(2, ko, p, a.shape[1])   # [2, KO, P, C]
    return np.ascontiguousarray(both.transpose(2, 0, 1, 3))


def _chains(a):
    """[P, 2, KO, DHG] -> [2, P, 2, 2, KO, 128] chain-pair blocks."""
    g = np.stack([a[:, :, :, c * P:(c + 1) * P] for c in range(4)], axis=0)
    g = g.reshape(2, 2, P, 2, KO, P)        # [pair, c2, P, r, KO, dh]
    return np.ascontiguousarray(g.transpose(0, 2, 1, 3, 4, 5))


def kernel(x, context, Wq, Wk, Wv, Wo, bo, **extra):
    nc = _get_nc()
    B = x.shape[0]
    zeros_bo = np.zeros((1, E), dtype=np.float32)
    bo_full = np.ascontiguousarray(np.asarray(bo, dtype=np.float32).reshape(1, E))
    ident = np.eye(P, dtype=np.float16)
    x = np.asarray(x, dtype=np.float32)
    context = np.asarray(context, dtype=np.float32)
    in_maps = []
    for c in range(8):
        b, g = c // 2, c % 2
        wq_s = np.asarray(Wq[:, g * DHG:(g + 1) * DHG], dtype=np.float32)[:, _PERM]
        wk_s = np.asarray(Wk[:, g * DHG:(g + 1) * DHG], dtype=np.float32)[:, _PERM]
        wv_s = np.asarray(Wv[:, g * DHG:(g + 1) * DHG], dtype=np.float32)
        wo_s = np.asarray(Wo[g * DHG:(g + 1) * DHG, :], dtype=np.float32)
        in_maps.append({
            "xp": _pair(np.ascontiguousarray(x[b].T), XS, KO, P),
            "cp": _pair(np.ascontiguousarray(context[b].T), XS, KO, P),
            "wqp": _chains(_pair(wq_s, WS, KO, P)),
            "wkp": _chains(_pair(wk_s, WS, KO, P)),
            "wvp": _pair(wv_s, WS, KO, P),
            "wop": _pair(wo_s, WS, DO, P),
            "ident": ident,
            "bo": (bo_full if g == 0 else zeros_bo),
        })
    global _last_in_maps
    _last_in_maps = in_maps
    res = run_bass_kernel_spmd(nc, in_maps, list(range(8)))
    out = np.empty((B, N, E), dtype=np.float32)
    for b in range(B):
        out[b] = res.results[2 * b]["out"].astype(np.float32) \
            + res.results[2 * b + 1]["out"].astype(np.float32)
    return out


# revision 50
# speedup vs baseline: 1.0472x; 1.0472x over previous
"""Cross-attention Trainium2 kernel (8 NeuronCores, SPMD).

Sharding: core c handles batch c//2 and head-group c%2 (8 of 16 heads).
Each core computes its head-group's partial output projection; the host
sums the two partials per batch (bias is folded into head-group 0).

Design (cost-model driven; see transcript):
- All inputs arrive HOST-TRANSPOSED in the exact SBUF layout (k-major),
  with fp8e4m3 main+residual pairs packed in one tensor per operand
  ([P, 2, KO, C]; dim1 = {e4m3(s*a), e4m3(s*a - rounded)}), so the
  critical first-exp DMA path is 4 transfers.
- Q/K/V/O projections run as fp8 DoubleRow with 3-chain residual
  compensation (a8@b8 + da8@b8 + a8@db8) accumulated in one PSUM tile:
  4x faster per chain than fp16, 3 chains -> 1.33x net, ~fp16 accuracy.
- QK^T runs as fp8 DoubleRow on 32-partition row tiles (contraction
  64 = 32 partitions x 2 half-head k-tiles interleaved in the free
  dim): 2x over fp16.  Q/K are requantized to e4m3 (x16) from the
  projection PSUM; the 1/256 descale folds into the exp scale.
- exp on ACT is the critical engine (~133us busy).  Emission is
  unit-granular: after every S PSUM tile (2 chunks + exp) the PE pops
  ~0.9us of queued work (PV chains, projection sub-chains, out-proj)
  so ACT never starves and the PE p-state stays warm (the cost model
  halves PE speed after ~3.4us of idle; dummy warm-up matmuls cover
  the DMA-bound head).
- PV stays fp16 (plain fp8 fails the 2e-2 gate).  Softmax row sums
  ride as a ones column in V; DVE normalizes with reciprocals.
- O round-trip: O_sb -> DRAM scratch -> XBAR transpose per 128-query
  block -> compensated-fp8 out-proj, interleaved into later slots.
  The last query chunk skips the DRAM round-trip entirely: each
  finished head pair is PE-transposed (identity matmul) straight into
  the fp8 out-proj operands, removing two serial DMA hops per block
  from the kernel tail.
"""
import sys

if "/opt/trn_rl_repo" not in sys.path:
    sys.path.insert(0, "/opt/trn_rl_repo")

from collections import deque

import numpy as np
import ml_dtypes

import concourse.bass as bass  # noqa: F401
import concourse.tile as tile
from concourse import bacc, mybir
from concourse.bass_utils import run_bass_kernel_spmd

P = 128
N = 2048          # queries per batch
M = 1024          # context rows
K = 1024          # query_dim == context_dim
DHG = 512         # d_attn per head group (8 heads x 64)
DH = 64           # dim per head
HL = 8            # heads per core
E = 1024          # output dim
SCALE = DH ** -0.5
QS = 16.0         # fp8 scale for q8/k8 and oT8
XS = 4.0          # host fp8 scale for x/ctx
WS = 64.0         # host fp8 scale for weights
F32 = mybir.dt.float32
F16 = mybir.dt.float16
E4 = mybir.dt.float8e4
E4NP = ml_dtypes.float8_e4m3

KO = K // P       # 8 contraction chunks
KP = KO // 2      # 4 DoubleRow contraction pairs
MT = M // P       # 8 context tiles
DO = DHG // P     # 4 head-dim chunks
QC = N // 512     # 4 query chunks of 512
NC = 512 // P     # 4 query sub-tiles per chunk
EC = E // 512     # 2 output chunks of 512

DR = mybir.MatmulPerfMode.DoubleRow
MUL = mybir.AluOpType.mult
ADD = mybir.AluOpType.add
SUB = mybir.AluOpType.subtract

_CACHE = {}


def _build():
    nc = bacc.Bacc("TRN2", target_bir_lowering=False, debug=False, num_devices=8)
    xp_d = nc.dram_tensor("xp", [P, 2, KO, N], E4, kind="ExternalInput")
    cp_d = nc.dram_tensor("cp", [P, 2, KO, M], E4, kind="ExternalInput")
    wqp_d = nc.dram_tensor("wqp", [2, P, 2, 2, KO, P], E4, kind="ExternalInput")
    wkp_d = nc.dram_tensor("wkp", [2, P, 2, 2, KO, P], E4, kind="ExternalInput")
    wvp_d = nc.dram_tensor("wvp", [P, 2, KO, DHG], E4, kind="ExternalInput")
    wop_d = nc.dram_tensor("wop", [P, 2, DO, E], E4, kind="ExternalInput")
    ident_d = nc.dram_tensor("ident", [P, P], F16, kind="ExternalInput")
    bo_d = nc.dram_tensor("bo", [1, E], F32, kind="ExternalInput")
    out_d = nc.dram_tensor("out", [N, E], F16, kind="ExternalOutput")
    oscr_d = nc.dram_tensor("oscr", [N, DHG], F16, kind="Internal")

    with tile.TileContext(nc) as tc:
        with tc.tile_pool(name="persist", bufs=1) as pp, \
             tc.tile_pool(name="ptp", bufs=8) as ptp, \
             tc.tile_pool(name="osb", bufs=2) as osb, \
             tc.tile_pool(name="otp", bufs=8) as otp, \
             tc.tile_pool(name="odp", bufs=4) as odp, \
             tc.tile_pool(name="psS", bufs=2, space="PSUM") as psS, \
             tc.tile_pool(name="psV", bufs=2, space="PSUM") as psV, \
             tc.tile_pool(name="psF", bufs=2, space="PSUM") as psF:
            xp = pp.tile([P, 2, KO, N], E4)
            cp = pp.tile([P, 2, KO, M], E4)
            wqp = [pp.tile([P, 2, 2, KO, P], E4, name=f"wq_{c}") for c in range(2)]
            wkp = [pp.tile([P, 2, 2, KO, P], E4, name=f"wk_{c}") for c in range(2)]
            wvp = pp.tile([P, 2, KO, DHG], E4)
            wop = pp.tile([P, 2, DO, E], E4)
            ident = pp.tile([P, P], F16)
            scratch = pp.tile([P, 512], F16)
            bo_sb = pp.tile([1, E], F32)
            bias_sb = pp.tile([P, E], F32)
            # fp8 Q/K, half-head interleaved: [p=(h4,dh), hg, half, n]
            q8 = pp.tile([P, 2, 2, N], E4)
            k8 = pp.tile([P, 2, 2, M], E4)
            v_sb = pp.tile([P, MT, HL, DH + 1], F16)
            rec_sb = pp.tile([P, QC, HL, NC], F32)

            # ---------------- DMA loads -------------------------------
            # Critical path to the first exp: K/Q mains first, residuals
            # right behind, everything else after.
            nc.sync.dma_start(wkp[0][:], wkp_d[0])
            nc.sync.dma_start(cp[:, 0, :, 0:512], cp_d[:, 0, :, 0:512])
            nc.sync.dma_start(wqp[0][:], wqp_d[0])
            nc.sync.dma_start(xp[:, 0, :, 0:512], xp_d[:, 0, :, 0:512])
            nc.sync.dma_start(cp[:, 1, :, 0:512], cp_d[:, 1, :, 0:512])
            nc.sync.dma_start(xp[:, 1, :, 0:512], xp_d[:, 1, :, 0:512])
            nc.sync.dma_start(cp[:, 0, :, 512:M], cp_d[:, 0, :, 512:M])
            nc.sync.dma_start(cp[:, 1, :, 512:M], cp_d[:, 1, :, 512:M])
            nc.sync.dma_start(wkp[1][:], wkp_d[1])
            nc.sync.dma_start(wqp[1][:], wqp_d[1])
            nc.sync.dma_start(wvp[:], wvp_d[:])
            for qc in range(1, QC):
                nc.sync.dma_start(xp[:, :, :, qc * 512:(qc + 1) * 512],
                                  xp_d[:, :, :, qc * 512:(qc + 1) * 512])
            nc.sync.dma_start(wop[:], wop_d[:])
            nc.sync.dma_start(bo_sb[:], bo_d[:])
            nc.sync.dma_start(ident[:], ident_d[:])
            nc.gpsimd.partition_broadcast(bias_sb[:], bo_sb[:])
            nc.vector.memset(v_sb[:, :, :, DH], 1.0)
            nc.vector.memset(scratch[:], 0.0)

            # PE p-state warm-up / keep-warm dummies (the cost model halves
            # matmul speed unless the PE has been continuously busy ~3us).
            wid = [0]

            def warm(n):
                for _ in range(n):
                    s = psS.tile([P, 2, 512], F32, tag="s",
                                 name=f"warm_{wid[0]}")
                    wid[0] += 1
                    nc.tensor.matmul(
                        s[:, 0], scratch[:, 0:P], scratch[:],
                        start=True, stop=True, skip_group_check=True)

            # ------------- compensated fp8 projection chains ----------
            def sub_chain(ps, lhs, rhs, first, last):
                for kp in range(KP):
                    nc.tensor.matmul(
                        ps, lhs(kp), rhs(kp),
                        start=(first and kp == 0), stop=(last and kp == KP - 1),
                        perf_mode=DR,
                    )

            def qk_chain_units(name, wt, xt, cc, x0, finish, pool=None):
                cell = {}
                wc = wt[cc // 256]
                c2 = (cc // P) % 2
                wl = lambda r: (lambda kp: wc[:, c2, r, 2 * kp:2 * kp + 2, :])
                xr = lambda r: (lambda kp: xt[:, r, 2 * kp:2 * kp + 2, x0:x0 + 512])

                def u1():
                    if pool is None:
                        cell["ps"] = psF.tile([P, 512], F32, tag="f", name=name)
                    else:
                        # phase-A only: borrow an S-pool tile so the four
                        # head-of-kernel chains don't serialize on psF bufs
                        cell["ps"] = psS.tile([P, 2, 512], F32, tag="s",
                                              name=name)[:, 0, :]
                    sub_chain(cell["ps"][:], wl(0), xr(0), True, False)
                u2 = lambda: sub_chain(cell["ps"][:], wl(1), xr(0), False, False)

                def u3():
                    sub_chain(cell["ps"][:], wl(0), xr(1), False, True)
                    finish(cell["ps"])
                return [(430, u1), (430, u2), (470, u3)]

            def q_units(qc, hg, half, pool=None):
                cc = (hg * 2 + half) * P

                def fin(ps):
                    nc.vector.tensor_scalar(
                        q8[:, hg, half, qc * 512:(qc + 1) * 512], ps[:],
                        QS / (XS * WS), None, MUL)
                return qk_chain_units(f"qc_{qc}{hg}{half}", wqp, xp,
                                      cc, qc * 512, fin, pool=pool)

            def k_units(hg, half, ms):
                cc = (hg * 2 + half) * P

                def fin(ps):
                    nc.vector.tensor_scalar(
                        k8[:, hg, half, ms * 512:(ms + 1) * 512], ps[:],
                        QS / (XS * WS), None, MUL)
                return qk_chain_units(f"kc_{hg}{half}{ms}", wkp, cp,
                                      cc, ms * 512, fin)

            def v_units(mo):
                cell = {}
                cl = lambda r: (lambda kp: cp[:, r, 2 * kp:2 * kp + 2,
                                              mo * P:(mo + 1) * P])
                wr = lambda r: (lambda kp: wvp[:, r, 2 * kp:2 * kp + 2, :])

                def u1():
                    cell["ps"] = psF.tile([P, 512], F32, tag="f", name=f"vc_{mo}")
                    sub_chain(cell["ps"][:], cl(0), wr(0), True, False)
                u2 = lambda: sub_chain(cell["ps"][:], cl(1), wr(0), False, False)

                def u3():
                    ps = cell["ps"]
                    sub_chain(ps[:], cl(0), wr(1), False, True)
                    nc.vector.tensor_scalar(
                        v_sb[:, mo, :, 0:DH],
                        ps[:].rearrange("p (h d) -> p h d", h=HL),
                        1.0 / (XS * WS), None, MUL)
                return [(430, u1), (430, u2), (470, u3)]

            # ---------------- attention -------------------------------
            def s_tile(qc, h, mp, ptile):
                hg, pb = h // 4, (h % 4) * 32
                q0 = qc * 512
                s = psS.tile([P, 2, 512], F32, tag="s", name=f"s_{qc}_{h}_{mp}")
                for k2 in range(2):
                    mo = 2 * mp + k2
                    nc.tensor.matmul(
                        s[:, k2],
                        k8[pb:pb + 32, hg, :, mo * P:(mo + 1) * P],
                        q8[pb:pb + 32, hg, :, q0:q0 + 512],
                        start=True, stop=True,
                        perf_mode=DR,
                        tile_position=(pb, 0),
                        skip_group_check=True,
                    )
                nc.scalar.activation(
                    ptile[:, 2 * mp:2 * mp + 2, :], s[:],
                    mybir.ActivationFunctionType.Exp,
                    scale=SCALE / (QS * QS),
                )

            O_tiles = {}

            def pv_units(qc, h, ptile):
                cell = {}

                def mk(nci):
                    def u():
                        if nci == 0:
                            cell["pv"] = psV.tile([P, NC, DH + 1], F32,
                                                  tag="pv", name=f"pv_{qc}_{h}")
                        pv = cell["pv"]
                        for mo in range(MT):
                            nc.tensor.matmul(
                                pv[:, nci, :],
                                ptile[:, mo, nci * P:(nci + 1) * P],
                                v_sb[:, mo, h, :],
                                start=(mo == 0), stop=(mo == MT - 1),
                                skip_group_check=True,
                            )
                        if nci == NC - 1:
                            rec = rec_sb[:, qc, h, :]
                            nc.vector.reciprocal(rec, pv[:, :, DH])
                            O_sb = O_tiles[qc]
                            for i in range(NC):
                                nc.vector.tensor_scalar(
                                    O_sb[:, h, i, :], pv[:, i, 0:DH],
                                    rec[:, i:i + 1], None, MUL)
                            if qc != QC - 1:
                                q0 = qc * 512
                                nc.sync.dma_start(
                                    oscr_d[q0:q0 + 512, h * DH:(h + 1) * DH]
                                    .rearrange("(a pn) c -> pn a c", pn=P),
                                    O_sb[:, h])
                    return u
                return [(220, mk(0)), (220, mk(1)), (220, mk(2)), (500, mk(3))]

            def oproj_tiles(qc, nci):
                return (
                    otp.tile([P, DO, P], E4, tag="ot8", name=f"oT8_{qc}_{nci}"),
                    otp.tile([P, DO, P], E4, tag="dot8", name=f"doT8_{qc}_{nci}"),
                    odp.tile([P, E], F16, tag="od", name=f"od_{qc}_{nci}"),
                )

            def oproj_chain_units(qc, nci, oT8, doT8, od):
                """Compensated fp8 out-proj: oT8@wo8 + doT8@wo8 + oT8@dwo8."""
                last = qc == QC - 1
                q0 = qc * 512

                def mk(ec):
                    def u():
                        ps = psF.tile([P, 512], F32, tag="f",
                                      name=f"f_{qc}_{nci}_{ec}")
                        wr = lambda r: (lambda dp: wop[:, r, 2 * dp:2 * dp + 2,
                                                       ec * 512:(ec + 1) * 512])
                        ol = lambda t: (lambda dp: t[:, 2 * dp:2 * dp + 2, :])
                        for ci, (lt, rr) in enumerate(
                                [(oT8, 0), (doT8, 0), (oT8, 1)]):
                            for dp in range(DO // 2):
                                nc.tensor.matmul(
                                    ps[:], ol(lt)(dp), wr(rr)(dp),
                                    start=(ci == 0 and dp == 0),
                                    stop=(ci == 2 and dp == DO // 2 - 1),
                                    perf_mode=DR)
                        nc.vector.scalar_tensor_tensor(
                            od[:, ec * 512:(ec + 1) * 512], ps[:],
                            1.0 / (QS * WS),
                            bias_sb[:, ec * 512:(ec + 1) * 512], MUL, ADD)
                        if last:
                            nc.sync.dma_start(
                                out_d[q0 + nci * P:q0 + (nci + 1) * P,
                                      ec * 512:(ec + 1) * 512],
                                od[:, ec * 512:(ec + 1) * 512])
                        elif ec == EC - 1:
                            nc.gpsimd.dma_start(
                                out_d[q0 + nci * P:q0 + (nci + 1) * P, :], od[:])
                    return u
                return [(680, mk(0)), (700, mk(1))]

            # ---------------- schedule --------------------------------
            urgent = deque()
            background = deque()
            state = {"v_left": MT}

            def emit_budget(ns):
                spent = 0
                while spent < ns and (urgent or background):
                    est, u = urgent.popleft() if urgent else background.popleft()
                    u()
                    spent += est
                if spent == 0:
                    # queues dry: keep the PE p-state warm
                    warm(1)

            def push_pv(qc, h, pt):
                urgent.extend(pv_units(qc, h, pt))
                last = qc == QC - 1
                if h == HL - 1 and not last:
                    def tail():
                        q0 = qc * 512
                        for nci in range(NC):
                            oT = otp.tile([P, DO, P], F16, tag="ot",
                                          name=f"oT_{qc}_{nci}")
                            nc.sync.dma_start_transpose(
                                oT[:], oscr_d[q0 + nci * P:q0 + (nci + 1) * P, :])
                            oT8, doT8, od = oproj_tiles(qc, nci)

                            def conv(oT=oT, oT8=oT8, doT8=doT8):
                                nc.vector.tensor_scalar(
                                    oT8[:], oT[:], QS, None, MUL)
                                nc.vector.scalar_tensor_tensor(
                                    doT8[:], oT[:], QS, oT8[:], MUL, SUB)
                            background.append((100, conv))
                            background.extend(
                                oproj_chain_units(qc, nci, oT8, doT8, od))
                    urgent.append((0, tail))
                if last and h in (1, 3, 5):
                    # qc3 skips the DRAM round-trip entirely: PE-transpose
                    # each finished head pair straight into oT8/doT8.
                    d = h // 2

                    def tp_nci(nci, d=d):
                        def u():
                            if d == 0:
                                state[f"o3_{nci}"] = oproj_tiles(qc, nci)
                            oT8, doT8, _ = state[f"o3_{nci}"]
                            ps = psF.tile([P, 512], F32, tag="f",
                                          name=f"tp{d}_{nci}")
                            pv16 = ps[:].bitcast(F16)
                            nc.tensor.transpose(
                                pv16[0:64, 0:P],
                                O_tiles[qc][:, 2 * d, nci, :], ident[:],
                                tile_position=(0, 0))
                            nc.tensor.transpose(
                                pv16[64:128, 0:P],
                                O_tiles[qc][:, 2 * d + 1, nci, :], ident[:],
                                tile_position=(0, 64))
                            nc.vector.tensor_scalar(
                                oT8[:, d, :], pv16[:, 0:P], QS, None, MUL)
                            nc.vector.scalar_tensor_tensor(
                                doT8[:, d, :], pv16[:, 0:P], QS,
                                oT8[:, d, :], MUL, SUB)
                        return u
                    for nci in range(NC):
                        urgent.append((250, tp_nci(nci)))
                if last and h == HL - 2:
                    def tail6():
                        for nci in range(NC):
                            oT8, doT8, _ = state[f"o3_{nci}"]
                            ps = psF.tile([P, 512], F32, tag="f",
                                          name=f"t6_{nci}")
                            pv16 = ps[:].bitcast(F16)
                            nc.tensor.transpose(
                                pv16[0:64, 0:P],
                                O_tiles[qc][:, 6, nci, :], ident[:],
                                tile_position=(0, 0))
                            nc.vector.tensor_scalar(
                                oT8[0:64, 3, :], pv16[0:64, 0:P],
                                QS, None, MUL)
                            nc.vector.scalar_tensor_tensor(
                                doT8[0:64, 3, :], pv16[0:64, 0:P], QS,
                                oT8[0:64, 3, :], MUL, SUB)
                    urgent.append((300, tail6))
                if last and h == HL - 1:
                    def tail7():
                        for nci in range(NC):
                            oT8, doT8, _ = state[f"o3_{nci}"]
                            ps = psF.tile([P, 512], F32, tag="f",
                                          name=f"t7_{nci}")
                            pv16 = ps[:].bitcast(F16)
                            nc.tensor.transpose(
                                pv16[64:128, 0:P],
                                O_tiles[qc][:, 7, nci, :], ident[:],
                                tile_position=(0, 64))
                            nc.vector.tensor_scalar(
                                oT8[64:128, 3, :], pv16[64:128, 0:P],
                                QS, None, MUL)
                            nc.vector.scalar_tensor_tensor(
                                doT8[64:128, 3, :], pv16[64:128, 0:P], QS,
                                oT8[64:128, 3, :], MUL, SUB)
                        for nci in range(NC):
                            oT8, doT8, od = state[f"o3_{nci}"]
                            urgent.extend(
                                oproj_chain_units(qc, nci, oT8, doT8, od))
                    urgent.append((0, tail7))

            def mk_v(mo):
                def f():
                    for est, u in v_units(mo):
                        u()
                    state["v_left"] -= 1
                return (1330, f)

            # phase A: minimum work before the first exp.  Sub-chain order
            # tracks DMA arrival: mains (u1, u2 use the weight pair + the
            # x/ctx main half), then the x/ctx-residual chains (u3).
            warm(8)
            ka, kb = k_units(0, 0, 0), k_units(0, 1, 0)
            qa, qb = q_units(0, 0, 0, pool="s"), q_units(0, 0, 1, pool="s")
            for est, u in [ka[0], ka[1], kb[0], kb[1]]:
                u()
                warm(1)
            for est, u in [qa[0], qa[1], qb[0], qb[1], ka[2], kb[2],
                           qa[2], qb[2]]:
                u()
            O_tiles[0] = osb.tile([P, HL, NC, DH], F16, tag="o", name="O_0")
            pt00 = ptp.tile([P, MT, 512], F16, tag="pt", name="pt_0_0")
            s_tile(0, 0, 0, pt00)
            for est, u in k_units(0, 0, 1) + k_units(0, 1, 1):
                u()
            s_tile(0, 0, 1, pt00)

            def marker(key):
                return (0, lambda: state.__setitem__(key, True))

            background.extend(k_units(1, 0, 0) + k_units(1, 1, 0))
            background.extend(q_units(0, 1, 0) + q_units(0, 1, 1))
            background.extend(k_units(1, 0, 1) + k_units(1, 1, 1))
            background.append(marker("hg1"))
            background.extend([mk_v(mo) for mo in range(MT)])

            def drain_until(key):
                while not state.get(key) and (urgent or background):
                    emit_budget(1)

            pv_pending = deque([(0, 0, pt00)])
            s_tile(0, 0, 2, pt00)
            emit_budget(900)
            s_tile(0, 0, 3, pt00)
            emit_budget(900)

            for s in range(1, QC * HL):
                qc, h = divmod(s, HL)
                if h == 0:
                    O_tiles[qc] = osb.tile([P, HL, NC, DH], F16, tag="o",
                                           name=f"O_{qc}")
                if h == 1 and qc + 1 < QC:
                    for hg in range(2):
                        for hf in range(2):
                            background.extend(q_units(qc + 1, hg, hf))
                    background.append(marker(f"q{qc + 1}"))
                # S(0, h>=4) needs the hg1 K/Q chains; S(qc, 0) needs the
                # q8 chains of qc -- force-drain them if the budget lagged.
                if qc == 0 and h == 4:
                    drain_until("hg1")
                if h == 0 and qc >= 1:
                    drain_until(f"q{qc}")
                pt = ptp.tile([P, MT, 512], F16, tag="pt", name=f"pt_{qc}_{h}")
                pv_pending.append((qc, h, pt))
                lag = 2 if s < 24 else 1
                while len(pv_pending) > lag and state["v_left"] == 0:
                    push_pv(*pv_pending.popleft())
                for mp in range(4):
                    s_tile(qc, h, mp, pt)
                    emit_budget(840)

            while pv_pending:
                push_pv(*pv_pending.popleft())
            while urgent or background:
                emit_budget(10000)
    nc.finalize()
    return nc


def _get_nc():
    if "nc" not in _CACHE:
        _CACHE["nc"] = _build()
    return _CACHE["nc"]


# column permutation for Wq/Wk: chain-major [hg, half, h4, dh] ordering
def _qk_perm():
    j = np.arange(DHG)
    hg, r = j // 256, j % 256
    half, r2 = r // 128, r % 128
    h4, dh = r2 // 32, r2 % 32
    return hg * 256 + h4 * 64 + half * 32 + dh


_PERM = _qk_perm()


def _pair(a, scale, ko, p):
    """[K, C] -> [P, 2, KO, C]: {e4m3(s*a), residual} in SBUF layout."""
    s = (np.asarray(a, dtype=np.float32) * scale)
    hi = s.astype(E4NP)
    lo = (s - hi.astype(np.float32)).astype(E4NP)
    both = np.stack([hi, lo], axis=0)           # [2, K, C]
    both = both.reshape(2, ko, p, a.shape[1])   # [2, KO, P, C]
    return np.ascontiguousarray(both.transpose(2, 0, 1, 3))


def _chains(a):
    """[P, 2, KO, DHG] -> [2, P, 2, 2, KO, 128] chain-pair blocks."""
    g = np.stack([a[:, :, :, c * P:(c + 1) * P] for c in range(4)], axis=0)
    g = g.reshape(2, 2, P, 2, KO, P)        # [pair, c2, P, r, KO, dh]
    return np.ascontiguousarray(g.transpose(0, 2, 1, 3, 4, 5))


def kernel(x, context, Wq, Wk, Wv, Wo, bo, **extra):
    nc = _get_nc()
    B = x.shape[0]
    zeros_bo = np.zeros((1, E), dtype=np.float32)
    bo_full = np.ascontiguousarray(np.asarray(bo, dtype=np.float32).reshape(1, E))
    ident = np.eye(P, dtype=np.float16)
    x = np.asarray(x, dtype=np.float32)
    context = np.asarray(context, dtype=np.float32)
    in_maps = []
    for c in range(8):
        b, g = c // 2, c % 2
        wq_s = np.asarray(Wq[:, g * DHG:(g + 1) * DHG], dtype=np.float32)[:, _PERM]
        wk_s = np.asarray(Wk[:, g * DHG:(g + 1) * DHG], dtype=np.float32)[:, _PERM]
        wv_s = np.asarray(Wv[:, g * DHG:(g + 1) * DHG], dtype=np.float32)
        wo_s = np.asarray(Wo[g * DHG:(g + 1) * DHG, :], dtype=np.float32)
        in_maps.append({
            "xp": _pair(np.ascontiguousarray(x[b].T), XS, KO, P),
            "cp": _pair(np.ascontiguousarray(context[b].T), XS, KO, P),
            "wqp": _chains(_pair(wq_s, WS, KO, P)),
            "wkp": _chains(_pair(wk_s, WS, KO, P)),
            "wvp": _pair(wv_s, WS, KO, P),
            "wop": _pair(wo_s, WS, DO, P),
            "ident": ident,
            "bo": (bo_full if g == 0 else zeros_bo),
        })
    global _last_in_maps
    _last_in_maps = in_maps
    res = run_bass_kernel_spmd(nc, in_maps, list(range(8)))
    out = np.empty((B, N, E), dtype=np.float32)
    for b in range(B):
        out[b] = res.results[2 * b]["out"].astype(np.float32) \
            + res.results[2 * b + 1]["out"].astype(np.float32)
    return out


# revision 61
# speedup vs baseline: 1.0608x; 1.0130x over previous
"""Cross-attention Trainium2 kernel (8 NeuronCores, SPMD).

Sharding: core c handles batch c//2 and head-group c%2 (8 of 16 heads).
Each core computes its head-group's partial output projection; the host
sums the two partials per batch (bias is folded into head-group 0).

Design (cost-model driven; see transcript):
- All inputs arrive HOST-TRANSPOSED in the exact SBUF layout (k-major),
  with fp8e4m3 main+residual pairs packed in one tensor per operand
  ([P, 2, KO, C]; dim1 = {e4m3(s*a), e4m3(s*a - rounded)}), so the
  critical first-exp DMA path is 4 transfers.
- Q/K/V/O projections run as fp8 DoubleRow with 3-chain residual
  compensation (a8@b8 + da8@b8 + a8@db8) accumulated in one PSUM tile:
  4x faster per chain than fp16, 3 chains -> 1.33x net, ~fp16 accuracy.
- QK^T runs as fp8 DoubleRow on 32-partition row tiles (contraction
  64 = 32 partitions x 2 half-head k-tiles interleaved in the free
  dim): 2x over fp16.  Q/K are requantized to e4m3 (x16) from the
  projection PSUM; the 1/256 descale folds into the exp scale.
- exp on ACT is the critical engine (~133us busy).  Emission is
  unit-granular: after every S PSUM tile (2 chunks + exp) the PE pops
  ~0.9us of queued work (PV chains, projection sub-chains, out-proj)
  so ACT never starves and the PE p-state stays warm (the cost model
  halves PE speed after ~3.4us of idle; dummy warm-up matmuls cover
  the DMA-bound head).
- PV stays fp16 (plain fp8 fails the 2e-2 gate).  Softmax row sums
  ride as a ones column in V; DVE normalizes with reciprocals.
- O round-trip: O_sb -> DRAM scratch -> XBAR transpose per 128-query
  block -> compensated-fp8 out-proj, interleaved into later slots.
  The last query chunk skips the DRAM round-trip entirely: each
  finished head pair is PE-transposed (identity matmul) straight into
  the fp8 out-proj operands, removing two serial DMA hops per block
  from the kernel tail.
"""
import sys

if "/opt/trn_rl_repo" not in sys.path:
    sys.path.insert(0, "/opt/trn_rl_repo")

from collections import deque

import numpy as np
import ml_dtypes

import concourse.bass as bass  # noqa: F401
import concourse.tile as tile
from concourse import bacc, mybir
from concourse.bass_utils import run_bass_kernel_spmd

P = 128
N = 2048          # queries per batch
M = 1024          # context rows
K = 1024          # query_dim == context_dim
DHG = 512         # d_attn per head group (8 heads x 64)
DH = 64           # dim per head
HL = 8            # heads per core
E = 1024          # output dim
SCALE = DH ** -0.5
QS = 16.0         # fp8 scale for q8/k8 and oT8
XS = 4.0          # host fp8 scale for x/ctx
WS = 64.0         # host fp8 scale for weights
F32 = mybir.dt.float32
F16 = mybir.dt.float16
E4 = mybir.dt.float8e4
E4NP = ml_dtypes.float8_e4m3

KO = K // P       # 8 contraction chunks
KP = KO // 2      # 4 DoubleRow contraction pairs
MT = M // P       # 8 context tiles
DO = DHG // P     # 4 head-dim chunks
QC = N // 512     # 4 query chunks of 512
NC = 512 // P     # 4 query sub-tiles per chunk
EC = E // 512     # 2 output chunks of 512

DR = mybir.MatmulPerfMode.DoubleRow
MUL = mybir.AluOpType.mult
ADD = mybir.AluOpType.add
SUB = mybir.AluOpType.subtract

_CACHE = {}


def _build():
    nc = bacc.Bacc("TRN2", target_bir_lowering=False, debug=False, num_devices=8)
    xp_d = nc.dram_tensor("xp", [P, 2, KO, N], E4, kind="ExternalInput")
    cp_d = nc.dram_tensor("cp", [P, 2, KO, M], E4, kind="ExternalInput")
    wqp_d = nc.dram_tensor("wqp", [2, P, 2, 2, KO, P], E4, kind="ExternalInput")
    wkp_d = nc.dram_tensor("wkp", [2, P, 2, 2, KO, P], E4, kind="ExternalInput")
    wvp_d = nc.dram_tensor("wvp", [P, 2, KO, DHG], E4, kind="ExternalInput")
    wop_d = nc.dram_tensor("wop", [P, 2, DO, E], E4, kind="ExternalInput")
    ident_d = nc.dram_tensor("ident", [P, P], F16, kind="ExternalInput")
    bo_d = nc.dram_tensor("bo", [1, E], F32, kind="ExternalInput")
    out_d = nc.dram_tensor("out", [N, E], F16, kind="ExternalOutput")
    oscr_d = nc.dram_tensor("oscr", [N, DHG], F16, kind="Internal")

    with tile.TileContext(nc) as tc:
        with tc.tile_pool(name="persist", bufs=1) as pp, \
             tc.tile_pool(name="ptp", bufs=8) as ptp, \
             tc.tile_pool(name="osb", bufs=2) as osb, \
             tc.tile_pool(name="otp", bufs=8) as otp, \
             tc.tile_pool(name="odp", bufs=4) as odp, \
             tc.tile_pool(name="psS", bufs=2, space="PSUM") as psS, \
             tc.tile_pool(name="psV", bufs=2, space="PSUM") as psV, \
             tc.tile_pool(name="psF", bufs=2, space="PSUM") as psF:
            xp = pp.tile([P, 2, KO, N], E4)
            cp = pp.tile([P, 2, KO, M], E4)
            wqp = [pp.tile([P, 2, 2, KO, P], E4, name=f"wq_{c}") for c in range(2)]
            wkp = [pp.tile([P, 2, 2, KO, P], E4, name=f"wk_{c}") for c in range(2)]
            wvp = pp.tile([P, 2, KO, DHG], E4)
            wop = pp.tile([P, 2, DO, E], E4)
            ident = pp.tile([P, P], F16)
            scratch = pp.tile([P, 512], F16)
            bo_sb = pp.tile([1, E], F32)
            bias_sb = pp.tile([P, E], F32)
            # fp8 Q/K, half-head interleaved: [p=(h4,dh), hg, half, n]
            q8 = pp.tile([P, 2, 2, N], E4)
            k8 = pp.tile([P, 2, 2, M], E4)
            v_sb = pp.tile([P, MT, HL, DH + 1], F16)
            rec_sb = pp.tile([P, QC, HL, NC], F32)

            # ---------------- DMA loads -------------------------------
            # Critical path to the first exp: K/Q mains first, residuals
            # right behind, everything else after.
            nc.sync.dma_start(wkp[0][:], wkp_d[0])
            nc.sync.dma_start(cp[:, 0, :, 0:512], cp_d[:, 0, :, 0:512])
            nc.sync.dma_start(wqp[0][:], wqp_d[0])
            nc.sync.dma_start(xp[:, 0, :, 0:512], xp_d[:, 0, :, 0:512])
            nc.sync.dma_start(cp[:, 1, :, 0:512], cp_d[:, 1, :, 0:512])
            nc.sync.dma_start(xp[:, 1, :, 0:512], xp_d[:, 1, :, 0:512])
            nc.sync.dma_start(cp[:, 0, :, 512:M], cp_d[:, 0, :, 512:M])
            nc.sync.dma_start(cp[:, 1, :, 512:M], cp_d[:, 1, :, 512:M])
            nc.sync.dma_start(wkp[1][:], wkp_d[1])
            nc.sync.dma_start(wqp[1][:], wqp_d[1])
            nc.sync.dma_start(wvp[:], wvp_d[:])
            for qc in range(1, QC):
                nc.sync.dma_start(xp[:, :, :, qc * 512:(qc + 1) * 512],
                                  xp_d[:, :, :, qc * 512:(qc + 1) * 512])
            nc.sync.dma_start(wop[:], wop_d[:])
            nc.sync.dma_start(bo_sb[:], bo_d[:])
            nc.sync.dma_start(ident[:], ident_d[:])
            nc.gpsimd.partition_broadcast(bias_sb[:], bo_sb[:])
            nc.vector.memset(v_sb[:, :, :, DH], 1.0)
            nc.vector.memset(scratch[:], 0.0)

            # PE p-state warm-up / keep-warm dummies (the cost model halves
            # matmul speed unless the PE has been continuously busy ~3us).
            wid = [0]

            def warm(n):
                for _ in range(n):
                    s = psS.tile([P, 2, 512], F32, tag="s",
                                 name=f"warm_{wid[0]}")
                    wid[0] += 1
                    nc.tensor.matmul(
                        s[:, 0], scratch[:, 0:P], scratch[:],
                        start=True, stop=True, skip_group_check=True)

            # ------------- compensated fp8 projection chains ----------
            def sub_chain(ps, lhs, rhs, first, last):
                for kp in range(KP):
                    nc.tensor.matmul(
                        ps, lhs(kp), rhs(kp),
                        start=(first and kp == 0), stop=(last and kp == KP - 1),
                        perf_mode=DR,
                    )

            def qk_chain_units(name, wt, xt, cc, x0, finish, pool=None):
                cell = {}
                wc = wt[cc // 256]
                c2 = (cc // P) % 2
                wl = lambda r: (lambda kp: wc[:, c2, r, 2 * kp:2 * kp + 2, :])
                xr = lambda r: (lambda kp: xt[:, r, 2 * kp:2 * kp + 2, x0:x0 + 512])

                def u1():
                    if pool is None:
                        cell["ps"] = psF.tile([P, 512], F32, tag="f", name=name)
                    else:
                        # phase-A only: borrow an S-pool tile so the four
                        # head-of-kernel chains don't serialize on psF bufs
                        cell["ps"] = psS.tile([P, 2, 512], F32, tag="s",
                                              name=name)[:, 0, :]
                    sub_chain(cell["ps"][:], wl(0), xr(0), True, False)
                u2 = lambda: sub_chain(cell["ps"][:], wl(1), xr(0), False, False)

                def u3():
                    sub_chain(cell["ps"][:], wl(0), xr(1), False, True)
                    finish(cell["ps"])
                return [(430, u1), (430, u2), (470, u3)]

            def q_units(qc, hg, half, pool=None):
                cc = (hg * 2 + half) * P

                def fin(ps):
                    nc.vector.tensor_scalar(
                        q8[:, hg, half, qc * 512:(qc + 1) * 512], ps[:],
                        QS / (XS * WS), None, MUL)
                return qk_chain_units(f"qc_{qc}{hg}{half}", wqp, xp,
                                      cc, qc * 512, fin, pool=pool)

            def k_units(hg, half, ms):
                cc = (hg * 2 + half) * P

                def fin(ps):
                    nc.vector.tensor_scalar(
                        k8[:, hg, half, ms * 512:(ms + 1) * 512], ps[:],
                        QS / (XS * WS), None, MUL)
                return qk_chain_units(f"kc_{hg}{half}{ms}", wkp, cp,
                                      cc, ms * 512, fin)

            def v_units(mo):
                cell = {}
                cl = lambda r: (lambda kp: cp[:, r, 2 * kp:2 * kp + 2,
                                              mo * P:(mo + 1) * P])
                wr = lambda r: (lambda kp: wvp[:, r, 2 * kp:2 * kp + 2, :])

                def u1():
                    cell["ps"] = psF.tile([P, 512], F32, tag="f", name=f"vc_{mo}")
                    sub_chain(cell["ps"][:], cl(0), wr(0), True, False)
                u2 = lambda: sub_chain(cell["ps"][:], cl(1), wr(0), False, False)

                def u3():
                    ps = cell["ps"]
                    sub_chain(ps[:], cl(0), wr(1), False, True)
                    nc.vector.tensor_scalar(
                        v_sb[:, mo, :, 0:DH],
                        ps[:].rearrange("p (h d) -> p h d", h=HL),
                        1.0 / (XS * WS), None, MUL)
                return [(430, u1), (430, u2), (470, u3)]

            # ---------------- attention -------------------------------
            def s_tile(qc, h, mp, ptile):
                hg, pb = h // 4, (h % 4) * 32
                q0 = qc * 512
                s = psS.tile([P, 2, 512], F32, tag="s", name=f"s_{qc}_{h}_{mp}")
                for k2 in range(2):
                    mo = 2 * mp + k2
                    nc.tensor.matmul(
                        s[:, k2],
                        k8[pb:pb + 32, hg, :, mo * P:(mo + 1) * P],
                        q8[pb:pb + 32, hg, :, q0:q0 + 512],
                        start=True, stop=True,
                        perf_mode=DR,
                        tile_position=(pb, 0),
                        skip_group_check=True,
                    )
                nc.scalar.activation(
                    ptile[:, 2 * mp:2 * mp + 2, :], s[:],
                    mybir.ActivationFunctionType.Exp,
                    scale=SCALE / (QS * QS),
                )

            O_tiles = {}

            def pv_units(qc, h, ptile):
                cell = {}

                def mk(nci):
                    def u():
                        if nci == 0:
                            cell["pv"] = psV.tile([P, NC, DH + 1], F32,
                                                  tag="pv", name=f"pv_{qc}_{h}")
                        pv = cell["pv"]
                        for mo in range(MT):
                            nc.tensor.matmul(
                                pv[:, nci, :],
                                ptile[:, mo, nci * P:(nci + 1) * P],
                                v_sb[:, mo, h, :],
                                start=(mo == 0), stop=(mo == MT - 1),
                                skip_group_check=True,
                            )
                        if nci == NC - 1:
                            rec = rec_sb[:, qc, h, :]
                            nc.vector.reciprocal(rec, pv[:, :, DH])
                            O_sb = O_tiles[qc]
                            for i in range(NC):
                                nc.vector.tensor_scalar(
                                    O_sb[:, h, i, :], pv[:, i, 0:DH],
                                    rec[:, i:i + 1], None, MUL)
                            if qc != QC - 1:
                                q0 = qc * 512
                                nc.sync.dma_start(
                                    oscr_d[q0:q0 + 512, h * DH:(h + 1) * DH]
                                    .rearrange("(a pn) c -> pn a c", pn=P),
                                    O_sb[:, h])
                    return u
                return [(220, mk(0)), (220, mk(1)), (220, mk(2)), (500, mk(3))]

            def oproj_tiles(qc, nci):
                return (
                    otp.tile([P, DO, P], E4, tag="ot8", name=f"oT8_{qc}_{nci}"),
                    otp.tile([P, DO, P], E4, tag="dot8", name=f"doT8_{qc}_{nci}"),
                    odp.tile([P, E], F16, tag="od", name=f"od_{qc}_{nci}"),
                )

            def oproj_chain_units(qc, nci, oT8, doT8, od):
                """Compensated fp8 out-proj: oT8@wo8 + doT8@wo8 + oT8@dwo8."""
                last = qc == QC - 1
                q0 = qc * 512

                def mk(ec):
                    def u():
                        ps = psF.tile([P, 512], F32, tag="f",
                                      name=f"f_{qc}_{nci}_{ec}")
                        wr = lambda r: (lambda dp: wop[:, r, 2 * dp:2 * dp + 2,
                                                       ec * 512:(ec + 1) * 512])
                        ol = lambda t: (lambda dp: t[:, 2 * dp:2 * dp + 2, :])
                        for ci, (lt, rr) in enumerate(
                                [(oT8, 0), (doT8, 0), (oT8, 1)]):
                            for dp in range(DO // 2):
                                nc.tensor.matmul(
                                    ps[:], ol(lt)(dp), wr(rr)(dp),
                                    start=(ci == 0 and dp == 0),
                                    stop=(ci == 2 and dp == DO // 2 - 1),
                                    perf_mode=DR)
                        nc.vector.scalar_tensor_tensor(
                            od[:, ec * 512:(ec + 1) * 512], ps[:],
                            1.0 / (QS * WS),
                            bias_sb[:, ec * 512:(ec + 1) * 512], MUL, ADD)
                        if last:
                            nc.sync.dma_start(
                                out_d[q0 + nci * P:q0 + (nci + 1) * P,
                                      ec * 512:(ec + 1) * 512],
                                od[:, ec * 512:(ec + 1) * 512])
                        elif ec == EC - 1:
                            nc.gpsimd.dma_start(
                                out_d[q0 + nci * P:q0 + (nci + 1) * P, :], od[:])
                    return u
                return [(680, mk(0)), (700, mk(1))]

            # ---------------- schedule --------------------------------
            urgent = deque()
            background = deque()
            state = {"v_left": MT}

            def emit_budget(ns):
                spent = 0
                while spent < ns and (urgent or background):
                    est, u = urgent.popleft() if urgent else background.popleft()
                    u()
                    spent += est
                if spent == 0:
                    # queues dry: keep the PE p-state warm
                    warm(1)

            def push_pv(qc, h, pt):
                urgent.extend(pv_units(qc, h, pt))
                last = qc == QC - 1
                if h == HL - 1 and not last:
                    def tail():
                        q0 = qc * 512
                        for nci in range(NC):
                            oT = otp.tile([P, DO, P], F16, tag="ot",
                                          name=f"oT_{qc}_{nci}")
                            nc.sync.dma_start_transpose(
                                oT[:], oscr_d[q0 + nci * P:q0 + (nci + 1) * P, :])
                            oT8, doT8, od = oproj_tiles(qc, nci)

                            def conv(oT=oT, oT8=oT8, doT8=doT8):
                                nc.vector.tensor_scalar(
                                    oT8[:], oT[:], QS, None, MUL)
                                nc.vector.scalar_tensor_tensor(
                                    doT8[:], oT[:], QS, oT8[:], MUL, SUB)
                            background.append((100, conv))
                            background.extend(
                                oproj_chain_units(qc, nci, oT8, doT8, od))
                    urgent.append((0, tail))
                if last and h in (1, 3, 5):
                    # qc3 skips the DRAM round-trip entirely: PE-transpose
                    # each finished head pair straight into oT8/doT8.
                    d = h // 2

                    def tp_nci(nci, d=d):
                        def u():
                            if d == 0:
                                state[f"o3_{nci}"] = oproj_tiles(qc, nci)
                            oT8, doT8, _ = state[f"o3_{nci}"]
                            ps = psF.tile([P, 512], F32, tag="f",
                                          name=f"tp{d}_{nci}")
                            pv16 = ps[:].bitcast(F16)
                            nc.tensor.transpose(
                                pv16[0:64, 0:P],
                                O_tiles[qc][:, 2 * d, nci, :], ident[:],
                                tile_position=(0, 0))
                            nc.tensor.transpose(
                                pv16[64:128, 0:P],
                                O_tiles[qc][:, 2 * d + 1, nci, :], ident[:],
                                tile_position=(0, 64))
                            nc.vector.tensor_scalar(
                                oT8[:, d, :], pv16[:, 0:P], QS, None, MUL)
                            nc.vector.scalar_tensor_tensor(
                                doT8[:, d, :], pv16[:, 0:P], QS,
                                oT8[:, d, :], MUL, SUB)
                        return u
                    for nci in range(NC):
                        urgent.append((250, tp_nci(nci)))
                if last and h == HL - 2:
                    def tail6():
                        for nci in range(NC):
                            oT8, doT8, _ = state[f"o3_{nci}"]
                            ps = psF.tile([P, 512], F32, tag="f",
                                          name=f"t6_{nci}")
                            pv16 = ps[:].bitcast(F16)
                            nc.tensor.transpose(
                                pv16[0:64, 0:P],
                                O_tiles[qc][:, 6, nci, :], ident[:],
                                tile_position=(0, 0))
                            nc.vector.tensor_scalar(
                                oT8[0:64, 3, :], pv16[0:64, 0:P],
                                QS, None, MUL)
                            nc.vector.scalar_tensor_tensor(
                                doT8[0:64, 3, :], pv16[0:64, 0:P], QS,
                                oT8[0:64, 3, :], MUL, SUB)
                    urgent.append((300, tail6))
                if last and h == HL - 1:
                    def tail7():
                        for nci in range(NC):
                            oT8, doT8, _ = state[f"o3_{nci}"]
                            ps = psF.tile([P, 512], F32, tag="f",
                                          name=f"t7_{nci}")
                            pv16 = ps[:].bitcast(F16)
                            nc.tensor.transpose(
                                pv16[64:128, 0:P],
                                O_tiles[qc][:, 7, nci, :], ident[:],
                                tile_position=(0, 64))
                            nc.vector.tensor_scalar(
                                oT8[64:128, 3, :], pv16[64:128, 0:P],
                                QS, None, MUL)
                            nc.vector.scalar_tensor_tensor(
                                doT8[64:128, 3, :], pv16[64:128, 0:P], QS,
                                oT8[64:128, 3, :], MUL, SUB)
                        for nci in range(NC):
                            oT8, doT8, od = state[f"o3_{nci}"]
                            urgent.extend(
                                oproj_chain_units(qc, nci, oT8, doT8, od))
                    urgent.append((0, tail7))

            def mk_v(mo):
                def f():
                    for est, u in v_units(mo):
                        u()
                    state["v_left"] -= 1
                return (1330, f)

            # phase A: minimum work before the first exp.  Sub-chain order
            # tracks DMA arrival: mains (u1, u2 use the weight pair + the
            # x/ctx main half), then the x/ctx-residual chains (u3).
            warm(8)
            ka, kb = k_units(0, 0, 0), k_units(0, 1, 0)
            qa, qb = q_units(0, 0, 0, pool="s"), q_units(0, 0, 1, pool="s")
            for est, u in [ka[0], ka[1], kb[0], kb[1]]:
                u()
                warm(1)
            for est, u in [qa[0], qa[1], qb[0], qb[1], ka[2], kb[2],
                           qa[2], qb[2]]:
                u()
            O_tiles[0] = osb.tile([P, HL, NC, DH], F16, tag="o", name="O_0")
            pt00 = ptp.tile([P, MT, 512], F16, tag="pt", name="pt_0_0")
            s_tile(0, 0, 0, pt00)
            for est, u in k_units(0, 0, 1) + k_units(0, 1, 1):
                u()
            s_tile(0, 0, 1, pt00)

            def marker(key):
                return (0, lambda: state.__setitem__(key, True))

            background.extend(k_units(1, 0, 0) + k_units(1, 1, 0))
            background.extend(q_units(0, 1, 0) + q_units(0, 1, 1))
            background.extend(k_units(1, 0, 1) + k_units(1, 1, 1))
            background.append(marker("hg1"))
            background.extend([mk_v(mo) for mo in range(MT)])

            def drain_until(key):
                while not state.get(key) and (urgent or background):
                    emit_budget(1)

            pv_pending = deque([(0, 0, pt00)])
            s_tile(0, 0, 2, pt00)
            emit_budget(900)
            s_tile(0, 0, 3, pt00)
            emit_budget(900)

            for s in range(1, QC * HL):
                qc, h = divmod(s, HL)
                if h == 0:
                    O_tiles[qc] = osb.tile([P, HL, NC, DH], F16, tag="o",
                                           name=f"O_{qc}")
                if h == 1 and qc + 1 < QC:
                    for hg in range(2):
                        for hf in range(2):
                            background.extend(q_units(qc + 1, hg, hf))
                    background.append(marker(f"q{qc + 1}"))
                # S(0, h>=4) needs the hg1 K/Q chains; S(qc, 0) needs the
                # q8 chains of qc -- force-drain them if the budget lagged.
                if qc == 0 and h == 4:
                    drain_until("hg1")
                if h == 0 and qc >= 1:
                    drain_until(f"q{qc}")
                pt = ptp.tile([P, MT, 512], F16, tag="pt", name=f"pt_{qc}_{h}")
                pv_pending.append((qc, h, pt))
                lag = 2 if s < 32 else 1
                while len(pv_pending) > lag and state["v_left"] == 0:
                    push_pv(*pv_pending.popleft())
                for mp in range(4):
                    s_tile(qc, h, mp, pt)
                    emit_budget(520)

            while pv_pending:
                push_pv(*pv_pending.popleft())
            while urgent or background:
                emit_budget(10000)
    nc.finalize()
    return nc


def _get_nc():
    if "nc" not in _CACHE:
        _CACHE["nc"] = _build()
    return _CACHE["nc"]


# column permutation for Wq/Wk: chain-major [hg, half, h4, dh] ordering
def _qk_perm():
    j = np.arange(DHG)
    hg, r = j // 256, j % 256
    half, r2 = r // 128, r % 128
    h4, dh = r2 // 32, r2 % 32
    return hg * 256 + h4 * 64 + half * 32 + dh


_PERM = _qk_perm()


def _pair(a, scale, ko, p):
    """[K, C] -> [P, 2, KO, C]: {e4m3(s*a), residual} in SBUF layout."""
    s = (np.asarray(a, dtype=np.float32) * scale)
    hi = s.astype(E4NP)
    lo = (s - hi.astype(np.float32)).astype(E4NP)
    both = np.stack([hi, lo], axis=0)           # [2, K, C]
    both = both.reshape(2, ko, p, a.shape[1])   # [2, KO, P, C]
    return np.ascontiguousarray(both.transpose(2, 0, 1, 3))


def _chains(a):
    """[P, 2, KO, DHG] -> [2, P, 2, 2, KO, 128] chain-pair blocks."""
    g = np.stack([a[:, :, :, c * P:(c + 1) * P] for c in range(4)], axis=0)
    g = g.reshape(2, 2, P, 2, KO, P)        # [pair, c2, P, r, KO, dh]
    return np.ascontiguousarray(g.transpose(0, 2, 1, 3, 4, 5))


def kernel(x, context, Wq, Wk, Wv, Wo, bo, **extra):
    nc = _get_nc()
    B = x.shape[0]
    zeros_bo = np.zeros((1, E), dtype=np.float32)
    bo_full = np.ascontiguousarray(np.asarray(bo, dtype=np.float32).reshape(1, E))
    ident = np.eye(P, dtype=np.float16)
    x = np.asarray(x, dtype=np.float32)
    context = np.asarray(context, dtype=np.float32)
    in_maps = []
    for c in range(8):
        b, g = c // 2, c % 2
        wq_s = np.asarray(Wq[:, g * DHG:(g + 1) * DHG], dtype=np.float32)[:, _PERM]
        wk_s = np.asarray(Wk[:, g * DHG:(g + 1) * DHG], dtype=np.float32)[:, _PERM]
        wv_s = np.asarray(Wv[:, g * DHG:(g + 1) * DHG], dtype=np.float32)
        wo_s = np.asarray(Wo[g * DHG:(g + 1) * DHG, :], dtype=np.float32)
        in_maps.append({
            "xp": _pair(np.ascontiguousarray(x[b].T), XS, KO, P),
            "cp": _pair(np.ascontiguousarray(context[b].T), XS, KO, P),
            "wqp": _chains(_pair(wq_s, WS, KO, P)),
            "wkp": _chains(_pair(wk_s, WS, KO, P)),
            "wvp": _pair(wv_s, WS, KO, P),
            "wop": _pair(wo_s, WS, DO, P),
            "ident": ident,
            "bo": (bo_full if g == 0 else zeros_bo),
        })
    global _last_in_maps
    _last_in_maps = in_maps
    res = run_bass_kernel_spmd(nc, in_maps, list(range(8)))
    out = np.empty((B, N, E), dtype=np.float32)
    for b in range(B):
        out[b] = res.results[2 * b]["out"].astype(np.float32) \
            + res.results[2 * b + 1]["out"].astype(np.float32)
    return out


# revision 62
# speedup vs baseline: 1.0648x; 1.0038x over previous
"""Cross-attention Trainium2 kernel (8 NeuronCores, SPMD).

Sharding: core c handles batch c//2 and head-group c%2 (8 of 16 heads).
Each core computes its head-group's partial output projection; the host
sums the two partials per batch (bias is folded into head-group 0).

Design (cost-model driven; see transcript):
- All inputs arrive HOST-TRANSPOSED in the exact SBUF layout (k-major),
  with fp8e4m3 main+residual pairs packed in one tensor per operand
  ([P, 2, KO, C]; dim1 = {e4m3(s*a), e4m3(s*a - rounded)}), so the
  critical first-exp DMA path is 4 transfers.
- Q/K/V/O projections run as fp8 DoubleRow with 3-chain residual
  compensation (a8@b8 + da8@b8 + a8@db8) accumulated in one PSUM tile:
  4x faster per chain than fp16, 3 chains -> 1.33x net, ~fp16 accuracy.
- QK^T runs as fp8 DoubleRow on 32-partition row tiles (contraction
  64 = 32 partitions x 2 half-head k-tiles interleaved in the free
  dim): 2x over fp16.  Q/K are requantized to e4m3 (x16) from the
  projection PSUM; the 1/256 descale folds into the exp scale.
- exp on ACT is the critical engine (~133us busy).  Emission is
  unit-granular: after every S PSUM tile (2 chunks + exp) the PE pops
  ~0.9us of queued work (PV chains, projection sub-chains, out-proj)
  so ACT never starves and the PE p-state stays warm (the cost model
  halves PE speed after ~3.4us of idle; dummy warm-up matmuls cover
  the DMA-bound head).
- PV stays fp16 (plain fp8 fails the 2e-2 gate).  Softmax row sums
  ride as a ones column in V; DVE normalizes with reciprocals.
- O round-trip: O_sb -> DRAM scratch -> XBAR transpose per 128-query
  block -> compensated-fp8 out-proj, interleaved into later slots.
  The last query chunk skips the DRAM round-trip entirely: each
  finished head pair is PE-transposed (identity matmul) straight into
  the fp8 out-proj operands, removing two serial DMA hops per block
  from the kernel tail.
"""
import sys

if "/opt/trn_rl_repo" not in sys.path:
    sys.path.insert(0, "/opt/trn_rl_repo")

from collections import deque

import numpy as np
import ml_dtypes

import concourse.bass as bass  # noqa: F401
import concourse.tile as tile
from concourse import bacc, mybir
from concourse.bass_utils import run_bass_kernel_spmd

P = 128
N = 2048          # queries per batch
M = 1024          # context rows
K = 1024          # query_dim == context_dim
DHG = 512         # d_attn per head group (8 heads x 64)
DH = 64           # dim per head
HL = 8            # heads per core
E = 1024          # output dim
SCALE = DH ** -0.5
QS = 16.0         # fp8 scale for q8/k8 and oT8
XS = 4.0          # host fp8 scale for x/ctx
WS = 64.0         # host fp8 scale for weights
F32 = mybir.dt.float32
F16 = mybir.dt.float16
E4 = mybir.dt.float8e4
E4NP = ml_dtypes.float8_e4m3

KO = K // P       # 8 contraction chunks
KP = KO // 2      # 4 DoubleRow contraction pairs
MT = M // P       # 8 context tiles
DO = DHG // P     # 4 head-dim chunks
QC = N // 512     # 4 query chunks of 512
NC = 512 // P     # 4 query sub-tiles per chunk
EC = E // 512     # 2 output chunks of 512

DR = mybir.MatmulPerfMode.DoubleRow
MUL = mybir.AluOpType.mult
ADD = mybir.AluOpType.add
SUB = mybir.AluOpType.subtract

_CACHE = {}


def _build():
    nc = bacc.Bacc("TRN2", target_bir_lowering=False, debug=False, num_devices=8)
    xp_d = nc.dram_tensor("xp", [P, 2, KO, N], E4, kind="ExternalInput")
    cp_d = nc.dram_tensor("cp", [P, 2, KO, M], E4, kind="ExternalInput")
    wqp_d = nc.dram_tensor("wqp", [2, P, 2, 2, KO, P], E4, kind="ExternalInput")
    wkp_d = nc.dram_tensor("wkp", [2, P, 2, 2, KO, P], E4, kind="ExternalInput")
    wvp_d = nc.dram_tensor("wvp", [P, 2, KO, DHG], E4, kind="ExternalInput")
    wop_d = nc.dram_tensor("wop", [P, 2, DO, E], E4, kind="ExternalInput")
    ident_d = nc.dram_tensor("ident", [P, P], F16, kind="ExternalInput")
    bo_d = nc.dram_tensor("bo", [1, E], F32, kind="ExternalInput")
    out_d = nc.dram_tensor("out", [N, E], F16, kind="ExternalOutput")
    oscr_d = nc.dram_tensor("oscr", [N, DHG], F16, kind="Internal")

    with tile.TileContext(nc) as tc:
        with tc.tile_pool(name="persist", bufs=1) as pp, \
             tc.tile_pool(name="ptp", bufs=8) as ptp, \
             tc.tile_pool(name="osb", bufs=2) as osb, \
             tc.tile_pool(name="otp", bufs=8) as otp, \
             tc.tile_pool(name="odp", bufs=4) as odp, \
             tc.tile_pool(name="psS", bufs=2, space="PSUM") as psS, \
             tc.tile_pool(name="psV", bufs=2, space="PSUM") as psV, \
             tc.tile_pool(name="psF", bufs=2, space="PSUM") as psF:
            xp = pp.tile([P, 2, KO, N], E4)
            cp = pp.tile([P, 2, KO, M], E4)
            wqp = [pp.tile([P, 2, 2, KO, P], E4, name=f"wq_{c}") for c in range(2)]
            wkp = [pp.tile([P, 2, 2, KO, P], E4, name=f"wk_{c}") for c in range(2)]
            wvp = pp.tile([P, 2, KO, DHG], E4)
            wop = pp.tile([P, 2, DO, E], E4)
            ident = pp.tile([P, P], F16)
            scratch = pp.tile([P, 512], F16)
            bo_sb = pp.tile([1, E], F32)
            bias_sb = pp.tile([P, E], F32)
            # fp8 Q/K, half-head interleaved: [p=(h4,dh), hg, half, n]
            q8 = pp.tile([P, 2, 2, N], E4)
            k8 = pp.tile([P, 2, 2, M], E4)
            v_sb = pp.tile([P, MT, HL, DH + 1], F16)
            rec_sb = pp.tile([P, QC, HL, NC], F32)

            # ---------------- DMA loads -------------------------------
            # Critical path to the first exp: K/Q mains first, residuals
            # right behind, everything else after.
            nc.sync.dma_start(wkp[0][:], wkp_d[0])
            nc.sync.dma_start(cp[:, 0, :, 0:512], cp_d[:, 0, :, 0:512])
            nc.sync.dma_start(wqp[0][:], wqp_d[0])
            nc.sync.dma_start(xp[:, 0, :, 0:512], xp_d[:, 0, :, 0:512])
            nc.sync.dma_start(cp[:, 1, :, 0:512], cp_d[:, 1, :, 0:512])
            nc.sync.dma_start(xp[:, 1, :, 0:512], xp_d[:, 1, :, 0:512])
            nc.sync.dma_start(cp[:, 0, :, 512:M], cp_d[:, 0, :, 512:M])
            nc.sync.dma_start(cp[:, 1, :, 512:M], cp_d[:, 1, :, 512:M])
            nc.sync.dma_start(wkp[1][:], wkp_d[1])
            nc.sync.dma_start(wqp[1][:], wqp_d[1])
            nc.sync.dma_start(wvp[:], wvp_d[:])
            for qc in range(1, QC):
                nc.sync.dma_start(xp[:, :, :, qc * 512:(qc + 1) * 512],
                                  xp_d[:, :, :, qc * 512:(qc + 1) * 512])
            nc.sync.dma_start(wop[:], wop_d[:])
            nc.sync.dma_start(bo_sb[:], bo_d[:])
            nc.sync.dma_start(ident[:], ident_d[:])
            nc.gpsimd.partition_broadcast(bias_sb[:], bo_sb[:])
            nc.vector.memset(v_sb[:, :, :, DH], 1.0)
            nc.vector.memset(scratch[:], 0.0)

            # PE p-state warm-up / keep-warm dummies (the cost model halves
            # matmul speed unless the PE has been continuously busy ~3us).
            wid = [0]

            def warm(n):
                for _ in range(n):
                    s = psS.tile([P, 2, 512], F32, tag="s",
                                 name=f"warm_{wid[0]}")
                    wid[0] += 1
                    nc.tensor.matmul(
                        s[:, 0], scratch[:, 0:P], scratch[:],
                        start=True, stop=True, skip_group_check=True)

            # ------------- compensated fp8 projection chains ----------
            def sub_chain(ps, lhs, rhs, first, last):
                for kp in range(KP):
                    nc.tensor.matmul(
                        ps, lhs(kp), rhs(kp),
                        start=(first and kp == 0), stop=(last and kp == KP - 1),
                        perf_mode=DR,
                    )

            def qk_chain_units(name, wt, xt, cc, x0, finish, pool=None):
                cell = {}
                wc = wt[cc // 256]
                c2 = (cc // P) % 2
                wl = lambda r: (lambda kp: wc[:, c2, r, 2 * kp:2 * kp + 2, :])
                xr = lambda r: (lambda kp: xt[:, r, 2 * kp:2 * kp + 2, x0:x0 + 512])

                def u1():
                    if pool is None:
                        cell["ps"] = psF.tile([P, 512], F32, tag="f", name=name)
                    else:
                        # phase-A only: borrow an S-pool tile so the four
                        # head-of-kernel chains don't serialize on psF bufs
                        cell["ps"] = psS.tile([P, 2, 512], F32, tag="s",
                                              name=name)[:, 0, :]
                    sub_chain(cell["ps"][:], wl(0), xr(0), True, False)
                u2 = lambda: sub_chain(cell["ps"][:], wl(1), xr(0), False, False)

                def u3():
                    sub_chain(cell["ps"][:], wl(0), xr(1), False, True)
                    finish(cell["ps"])
                return [(430, u1), (430, u2), (470, u3)]

            def q_units(qc, hg, half, pool=None):
                cc = (hg * 2 + half) * P

                def fin(ps):
                    nc.vector.tensor_scalar(
                        q8[:, hg, half, qc * 512:(qc + 1) * 512], ps[:],
                        QS / (XS * WS), None, MUL)
                return qk_chain_units(f"qc_{qc}{hg}{half}", wqp, xp,
                                      cc, qc * 512, fin, pool=pool)

            def k_units(hg, half, ms):
                cc = (hg * 2 + half) * P

                def fin(ps):
                    nc.vector.tensor_scalar(
                        k8[:, hg, half, ms * 512:(ms + 1) * 512], ps[:],
                        QS / (XS * WS), None, MUL)
                return qk_chain_units(f"kc_{hg}{half}{ms}", wkp, cp,
                                      cc, ms * 512, fin)

            def v_units(mo):
                cell = {}
                cl = lambda r: (lambda kp: cp[:, r, 2 * kp:2 * kp + 2,
                                              mo * P:(mo + 1) * P])
                wr = lambda r: (lambda kp: wvp[:, r, 2 * kp:2 * kp + 2, :])

                def u1():
                    cell["ps"] = psF.tile([P, 512], F32, tag="f", name=f"vc_{mo}")
                    sub_chain(cell["ps"][:], cl(0), wr(0), True, False)
                u2 = lambda: sub_chain(cell["ps"][:], cl(1), wr(0), False, False)

                def u3():
                    ps = cell["ps"]
                    sub_chain(ps[:], cl(0), wr(1), False, True)
                    nc.vector.tensor_scalar(
                        v_sb[:, mo, :, 0:DH],
                        ps[:].rearrange("p (h d) -> p h d", h=HL),
                        1.0 / (XS * WS), None, MUL)
                return [(430, u1), (430, u2), (470, u3)]

            # ---------------- attention -------------------------------
            def s_tile(qc, h, mp, ptile):
                hg, pb = h // 4, (h % 4) * 32
                q0 = qc * 512
                s = psS.tile([P, 2, 512], F32, tag="s", name=f"s_{qc}_{h}_{mp}")
                for k2 in range(2):
                    mo = 2 * mp + k2
                    nc.tensor.matmul(
                        s[:, k2],
                        k8[pb:pb + 32, hg, :, mo * P:(mo + 1) * P],
                        q8[pb:pb + 32, hg, :, q0:q0 + 512],
                        start=True, stop=True,
                        perf_mode=DR,
                        tile_position=(pb, 0),
                        skip_group_check=True,
                    )
                nc.scalar.activation(
                    ptile[:, 2 * mp:2 * mp + 2, :], s[:],
                    mybir.ActivationFunctionType.Exp,
                    scale=SCALE / (QS * QS),
                )

            O_tiles = {}

            def pv_units(qc, h, ptile):
                cell = {}

                def mk(nci):
                    def u():
                        if nci == 0:
                            cell["pv"] = psV.tile([P, NC, DH + 1], F32,
                                                  tag="pv", name=f"pv_{qc}_{h}")
                        pv = cell["pv"]
                        for mo in range(MT):
                            nc.tensor.matmul(
                                pv[:, nci, :],
                                ptile[:, mo, nci * P:(nci + 1) * P],
                                v_sb[:, mo, h, :],
                                start=(mo == 0), stop=(mo == MT - 1),
                                skip_group_check=True,
                            )
                        if nci == NC - 1:
                            rec = rec_sb[:, qc, h, :]
                            nc.vector.reciprocal(rec, pv[:, :, DH])
                            O_sb = O_tiles[qc]
                            for i in range(NC):
                                nc.vector.tensor_scalar(
                                    O_sb[:, h, i, :], pv[:, i, 0:DH],
                                    rec[:, i:i + 1], None, MUL)
                            if qc != QC - 1:
                                q0 = qc * 512
                                nc.sync.dma_start(
                                    oscr_d[q0:q0 + 512, h * DH:(h + 1) * DH]
                                    .rearrange("(a pn) c -> pn a c", pn=P),
                                    O_sb[:, h])
                    return u
                return [(220, mk(0)), (220, mk(1)), (220, mk(2)), (500, mk(3))]

            def oproj_tiles(qc, nci):
                return (
                    otp.tile([P, DO, P], E4, tag="ot8", name=f"oT8_{qc}_{nci}"),
                    otp.tile([P, DO, P], E4, tag="dot8", name=f"doT8_{qc}_{nci}"),
                    odp.tile([P, E], F16, tag="od", name=f"od_{qc}_{nci}"),
                )

            def oproj_chain_units(qc, nci, oT8, doT8, od):
                """Compensated fp8 out-proj: oT8@wo8 + doT8@wo8 + oT8@dwo8."""
                last = qc == QC - 1
                q0 = qc * 512

                def mk(ec):
                    def u():
                        ps = psF.tile([P, 512], F32, tag="f",
                                      name=f"f_{qc}_{nci}_{ec}")
                        wr = lambda r: (lambda dp: wop[:, r, 2 * dp:2 * dp + 2,
                                                       ec * 512:(ec + 1) * 512])
                        ol = lambda t: (lambda dp: t[:, 2 * dp:2 * dp + 2, :])
                        for ci, (lt, rr) in enumerate(
                                [(oT8, 0), (doT8, 0), (oT8, 1)]):
                            for dp in range(DO // 2):
                                nc.tensor.matmul(
                                    ps[:], ol(lt)(dp), wr(rr)(dp),
                                    start=(ci == 0 and dp == 0),
                                    stop=(ci == 2 and dp == DO // 2 - 1),
                                    perf_mode=DR)
                        nc.vector.scalar_tensor_tensor(
                            od[:, ec * 512:(ec + 1) * 512], ps[:],
                            1.0 / (QS * WS),
                            bias_sb[:, ec * 512:(ec + 1) * 512], MUL, ADD)
                        if last:
                            nc.sync.dma_start(
                                out_d[q0 + nci * P:q0 + (nci + 1) * P,
                                      ec * 512:(ec + 1) * 512],
                                od[:, ec * 512:(ec + 1) * 512])
                        elif ec == EC - 1:
                            nc.gpsimd.dma_start(
                                out_d[q0 + nci * P:q0 + (nci + 1) * P, :], od[:])
                    return u
                return [(680, mk(0)), (700, mk(1))]

            # ---------------- schedule --------------------------------
            urgent = deque()
            background = deque()
            state = {"v_left": MT}

            def emit_budget(ns):
                spent = 0
                while spent < ns and (urgent or background):
                    est, u = urgent.popleft() if urgent else background.popleft()
                    u()
                    spent += est
                if spent == 0:
                    # queues dry: keep the PE p-state warm
                    warm(1)

            def push_pv(qc, h, pt):
                urgent.extend(pv_units(qc, h, pt))
                last = qc == QC - 1
                if h == HL - 1 and not last:
                    def tail():
                        q0 = qc * 512
                        for nci in range(NC):
                            oT = otp.tile([P, DO, P], F16, tag="ot",
                                          name=f"oT_{qc}_{nci}")
                            nc.sync.dma_start_transpose(
                                oT[:], oscr_d[q0 + nci * P:q0 + (nci + 1) * P, :])
                            oT8, doT8, od = oproj_tiles(qc, nci)

                            def conv(oT=oT, oT8=oT8, doT8=doT8):
                                nc.vector.tensor_scalar(
                                    oT8[:], oT[:], QS, None, MUL)
                                nc.vector.scalar_tensor_tensor(
                                    doT8[:], oT[:], QS, oT8[:], MUL, SUB)
                            background.append((100, conv))
                            background.extend(
                                oproj_chain_units(qc, nci, oT8, doT8, od))
                    urgent.append((0, tail))
                if last and h in (1, 3, 5):
                    # qc3 skips the DRAM round-trip entirely: PE-transpose
                    # each finished head pair straight into oT8/doT8.
                    d = h // 2

                    def tp_nci(nci, d=d):
                        def u():
                            if d == 0:
                                state[f"o3_{nci}"] = oproj_tiles(qc, nci)
                            oT8, doT8, _ = state[f"o3_{nci}"]
                            ps = psF.tile([P, 512], F32, tag="f",
                                          name=f"tp{d}_{nci}")
                            pv16 = ps[:].bitcast(F16)
                            nc.tensor.transpose(
                                pv16[0:64, 0:P],
                                O_tiles[qc][:, 2 * d, nci, :], ident[:],
                                tile_position=(0, 0))
                            nc.tensor.transpose(
                                pv16[64:128, 0:P],
                                O_tiles[qc][:, 2 * d + 1, nci, :], ident[:],
                                tile_position=(0, 64))
                            nc.vector.tensor_scalar(
                                oT8[:, d, :], pv16[:, 0:P], QS, None, MUL)
                            nc.vector.scalar_tensor_tensor(
                                doT8[:, d, :], pv16[:, 0:P], QS,
                                oT8[:, d, :], MUL, SUB)
                        return u
                    for nci in range(NC):
                        urgent.append((250, tp_nci(nci)))
                if last and h == HL - 2:
                    def tail6():
                        for nci in range(NC):
                            oT8, doT8, _ = state[f"o3_{nci}"]
                            ps = psF.tile([P, 512], F32, tag="f",
                                          name=f"t6_{nci}")
                            pv16 = ps[:].bitcast(F16)
                            nc.tensor.transpose(
                                pv16[0:64, 0:P],
                                O_tiles[qc][:, 6, nci, :], ident[:],
                                tile_position=(0, 0))
                            nc.vector.tensor_scalar(
                                oT8[0:64, 3, :], pv16[0:64, 0:P],
                                QS, None, MUL)
                            nc.vector.scalar_tensor_tensor(
                                doT8[0:64, 3, :], pv16[0:64, 0:P], QS,
                                oT8[0:64, 3, :], MUL, SUB)
                    urgent.append((300, tail6))
                if last and h == HL - 1:
                    def tail7():
                        for nci in range(NC):
                            oT8, doT8, _ = state[f"o3_{nci}"]
                            ps = psF.tile([P, 512], F32, tag="f",
                                          name=f"t7_{nci}")
                            pv16 = ps[:].bitcast(F16)
                            nc.tensor.transpose(
                                pv16[64:128, 0:P],
                                O_tiles[qc][:, 7, nci, :], ident[:],
                                tile_position=(0, 64))
                            nc.vector.tensor_scalar(
                                oT8[64:128, 3, :], pv16[64:128, 0:P],
                                QS, None, MUL)
                            nc.vector.scalar_tensor_tensor(
                                doT8[64:128, 3, :], pv16[64:128, 0:P], QS,
                                oT8[64:128, 3, :], MUL, SUB)
                        for nci in range(NC):
                            oT8, doT8, od = state[f"o3_{nci}"]
                            urgent.extend(
                                oproj_chain_units(qc, nci, oT8, doT8, od))
                    urgent.append((0, tail7))

            def mk_v(mo):
                us = v_units(mo)

                def last():
                    us[2][1]()
                    state["v_left"] -= 1
                return [us[0], us[1], (us[2][0], last)]

            # phase A: minimum work before the first exp.  Sub-chain order
            # tracks DMA arrival: mains (u1, u2 use the weight pair + the
            # x/ctx main half), then the x/ctx-residual chains (u3).
            warm(8)
            ka, kb = k_units(0, 0, 0), k_units(0, 1, 0)
            qa, qb = q_units(0, 0, 0, pool="s"), q_units(0, 0, 1, pool="s")
            for est, u in [ka[0], ka[1], kb[0], kb[1]]:
                u()
                warm(1)
            for est, u in [qa[0], qa[1], qb[0], qb[1], ka[2], kb[2],
                           qa[2], qb[2]]:
                u()
            O_tiles[0] = osb.tile([P, HL, NC, DH], F16, tag="o", name="O_0")
            pt00 = ptp.tile([P, MT, 512], F16, tag="pt", name="pt_0_0")
            s_tile(0, 0, 0, pt00)
            for est, u in k_units(0, 0, 1) + k_units(0, 1, 1):
                u()
            s_tile(0, 0, 1, pt00)

            def marker(key):
                return (0, lambda: state.__setitem__(key, True))

            background.extend(k_units(1, 0, 0) + k_units(1, 1, 0))
            background.extend(q_units(0, 1, 0) + q_units(0, 1, 1))
            background.extend(k_units(1, 0, 1) + k_units(1, 1, 1))
            background.append(marker("hg1"))
            for mo in range(MT):
                background.extend(mk_v(mo))

            def drain_until(key):
                while not state.get(key) and (urgent or background):
                    emit_budget(1)

            pv_pending = deque([(0, 0, pt00)])
            s_tile(0, 0, 2, pt00)
            emit_budget(900)
            s_tile(0, 0, 3, pt00)
            emit_budget(900)

            for s in range(1, QC * HL):
                qc, h = divmod(s, HL)
                if h == 0:
                    O_tiles[qc] = osb.tile([P, HL, NC, DH], F16, tag="o",
                                           name=f"O_{qc}")
                if h == 1 and qc + 1 < QC:
                    for hg in range(2):
                        for hf in range(2):
                            background.extend(q_units(qc + 1, hg, hf))
                    background.append(marker(f"q{qc + 1}"))
                # S(0, h>=4) needs the hg1 K/Q chains; S(qc, 0) needs the
                # q8 chains of qc -- force-drain them if the budget lagged.
                if qc == 0 and h == 4:
                    drain_until("hg1")
                if h == 0 and qc >= 1:
                    drain_until(f"q{qc}")
                pt = ptp.tile([P, MT, 512], F16, tag="pt", name=f"pt_{qc}_{h}")
                pv_pending.append((qc, h, pt))
                lag = 2 if s < 32 else 1
                while len(pv_pending) > lag and state["v_left"] == 0:
                    push_pv(*pv_pending.popleft())
                for mp in range(4):
                    s_tile(qc, h, mp, pt)
                    emit_budget(520)

            while pv_pending:
                push_pv(*pv_pending.popleft())
            while urgent or background:
                emit_budget(10000)
    nc.finalize()
    return nc


def _get_nc():
    if "nc" not in _CACHE:
        _CACHE["nc"] = _build()
    return _CACHE["nc"]


# column permutation for Wq/Wk: chain-major [hg, half, h4, dh] ordering
def _qk_perm():
    j = np.arange(DHG)
    hg, r = j // 256, j % 256
    half, r2 = r // 128, r % 128
    h4, dh = r2 // 32, r2 % 32
    return hg * 256 + h4 * 64 + half * 32 + dh


_PERM = _qk_perm()


def _pair(a, scale, ko, p):
    """[K, C] -> [P, 2, KO, C]: {e4m3(s*a), residual} in SBUF layout."""
    s = (np.asarray(a, dtype=np.float32) * scale)
    hi = s.astype(E4NP)
    lo = (s - hi.astype(np.float32)).astype(E4NP)
    both = np.stack([hi, lo], axis=0)           # [2, K, C]
    both = both.reshape(2, ko, p, a.shape[1])   # [2, KO, P, C]
    return np.ascontiguousarray(both.transpose(2, 0, 1, 3))


def _chains(a):
    """[P, 2, KO, DHG] -> [2, P, 2, 2, KO, 128] chain-pair blocks."""
    g = np.stack([a[:, :, :, c * P:(c + 1) * P] for c in range(4)], axis=0)
    g = g.reshape(2, 2, P, 2, KO, P)        # [pair, c2, P, r, KO, dh]
    return np.ascontiguousarray(g.transpose(0, 2, 1, 3, 4, 5))


def kernel(x, context, Wq, Wk, Wv, Wo, bo, **extra):
    nc = _get_nc()
    B = x.shape[0]
    zeros_bo = np.zeros((1, E), dtype=np.float32)
    bo_full = np.ascontiguousarray(np.asarray(bo, dtype=np.float32).reshape(1, E))
    ident = np.eye(P, dtype=np.float16)
    x = np.asarray(x, dtype=np.float32)
    context = np.asarray(context, dtype=np.float32)
    in_maps = []
    for c in range(8):
        b, g = c // 2, c % 2
        wq_s = np.asarray(Wq[:, g * DHG:(g + 1) * DHG], dtype=np.float32)[:, _PERM]
        wk_s = np.asarray(Wk[:, g * DHG:(g + 1) * DHG], dtype=np.float32)[:, _PERM]
        wv_s = np.asarray(Wv[:, g * DHG:(g + 1) * DHG], dtype=np.float32)
        wo_s = np.asarray(Wo[g * DHG:(g + 1) * DHG, :], dtype=np.float32)
        in_maps.append({
            "xp": _pair(np.ascontiguousarray(x[b].T), XS, KO, P),
            "cp": _pair(np.ascontiguousarray(context[b].T), XS, KO, P),
            "wqp": _chains(_pair(wq_s, WS, KO, P)),
            "wkp": _chains(_pair(wk_s, WS, KO, P)),
            "wvp": _pair(wv_s, WS, KO, P),
            "wop": _pair(wo_s, WS, DO, P),
            "ident": ident,
            "bo": (bo_full if g == 0 else zeros_bo),
        })
    global _last_in_maps
    _last_in_maps = in_maps
    res = run_bass_kernel_spmd(nc, in_maps, list(range(8)))
    out = np.empty((B, N, E), dtype=np.float32)
    for b in range(B):
        out[b] = res.results[2 * b]["out"].astype(np.float32) \
            + res.results[2 * b + 1]["out"].astype(np.float32)
    return out
